# revision 1
# baseline (speedup 1.0000x reference)
"""DeepseekV32 MLA-style attention on 8 Trainium2 NeuronCores (Bass/Tile).

Sharding: tensor-parallel over the 16 heads (2 heads per core) for
kv_b/attention; the S=2048 sequence is row-sharded (256 rows per core) for
the low-rank A projections and q_b; AllGather redistributes the compressed
KV, AllToAll redistributes q (row-shard -> head-shard) and the attention
output (head-shard -> row-shard); o_proj runs row-sharded so the final
output needs no reduction.

All matmuls run in bf16 with fp32 PSUM accumulation; softmax and rmsnorm
statistics are computed in fp32.
"""
import sys

sys.path.insert(0, "/opt/trn_rl_repo")

import numpy as np
import ml_dtypes
from contextlib import ExitStack

import concourse.bass as bass
import concourse.tile as tile
import concourse.mybir as mybir
from concourse import bacc
from concourse.masks import make_identity
from concourse.bass_utils import run_bass_kernel_spmd

BF16 = mybir.dt.bfloat16
F32 = mybir.dt.float32
AF = mybir.ActivationFunctionType

NC = 8            # cores
B, S, H = 1, 2048, 2048
NH = 16           # heads
QLR = 1536        # q lora rank
KVLR = 512        # kv lora rank
DR = 64           # rope dim
DN = 128          # nope dim
DV = 128          # v dim
DQK = DN + DR     # 192
EPS = 1e-6
HPC = NH // NC    # heads per core = 2
SPC = S // NC     # seq rows per core = 256
ST = SPC // 128   # row tiles per core = 2
NEG = -1e30       # causal mask fill

_CACHED = {}


def _ts(i, n):
    return slice(i * n, (i + 1) * n)


def build(use_collectives=True):
    nc = bacc.Bacc("TRN2", target_bir_lowering=False, debug=False, num_devices=NC)

    def collective(kind, in_ap, out_ap):
        if use_collectives:
            nc.gpsimd.collective_compute(
                kind, mybir.AluOpType.bypass, replica_groups=[list(range(NC))],
                ins=[in_ap.opt()], outs=[out_ap.opt()])
        else:
            # debug mode: local copy into the rank-0 slot (numerically wrong,
            # exercises everything but the collective)
            n = in_ap.shape[0]
            nc.sync.dma_start(out=out_ap[0:n], in_=in_ap)

    # ---- kernel I/O (per-core shards / replicated weights) ----
    hs_d = nc.dram_tensor("hs", [SPC, H], BF16, kind="ExternalInput").ap()
    cos_d = nc.dram_tensor("cosr", [SPC, DR], F32, kind="ExternalInput").ap()
    sin_d = nc.dram_tensor("sinr", [SPC, DR], F32, kind="ExternalInput").ap()
    wqa_d = nc.dram_tensor("wqa", [H, QLR], BF16, kind="ExternalInput").ap()
    wkva_d = nc.dram_tensor("wkva", [H, KVLR + DR], BF16, kind="ExternalInput").ap()
    wqb_d = nc.dram_tensor("wqb", [QLR, NH * DQK], BF16, kind="ExternalInput").ap()
    wkvb_d = nc.dram_tensor("wkvb", [KVLR, HPC * (DN + DV)], BF16,
                            kind="ExternalInput").ap()
    wo_d = nc.dram_tensor("wo", [NH * DV, H], BF16, kind="ExternalInput").ap()
    out_d = nc.dram_tensor("out", [SPC, H], F32, kind="ExternalOutput").ap()

    # ---- collective buffers ----
    # X = [ckv_normed^T (512) ; roped k_rot^T (64)] per core, gathered over S
    ag_in = nc.dram_tensor("ag_in", [KVLR + DR, SPC], BF16).ap()
    ag_out = nc.dram_tensor("ag_out", [NC * (KVLR + DR), SPC], BF16,
                            addr_space="Shared").ap()
    # q: row-shard [256, all 16 heads] -> head-shard [2048, 2 heads]
    a2aq_in = nc.dram_tensor("a2aq_in", [S, HPC * DQK], BF16).ap()
    a2aq_out = nc.dram_tensor("a2aq_out", [S, HPC * DQK], BF16).ap()
    # attn out (transposed): per-head AllToAll tensors declared in-line

    rg = [list(range(NC))]

    with tile.TileContext(nc) as tc, ExitStack() as ctx:
        # ---------------- pools ----------------
        singles = ctx.enter_context(tc.tile_pool(name="singles", bufs=1))
        small = ctx.enter_context(tc.tile_pool(name="small", bufs=4))
        cps = ctx.enter_context(tc.tile_pool(name="cps", bufs=4))  # psum copies
        interph = ctx.enter_context(tc.tile_pool(name="interph", bufs=1))

        ident = singles.tile([128, 128], BF16)
        make_identity(nc, ident)
        # causal additive mask for the diagonal 128x128 block:
        # mask[q, k] = 0 if k <= q else NEG
        eps_t = singles.tile([128, 1], F32)
        nc.vector.memset(eps_t, float(EPS))
        cmask = singles.tile([128, 128], F32)
        nc.gpsimd.memset(cmask, 0.0)
        nc.gpsimd.affine_select(
            out=cmask, in_=cmask, compare_op=mybir.AluOpType.is_ge,
            fill=NEG, base=0, pattern=[[-1, 128]], channel_multiplier=1)

        # rope cos/sin, repeated per head for the fused q-rope op
        # ce/co: [128, NH, 32] (head-broadcast), plus plain [128, 64] for k
        cos_sb, sin_sb = [], []
        cosq, sinq = [], []
        for st in range(ST):
            c_t = small.tile([128, DR], F32, tag="cos", bufs=2)
            s_t = small.tile([128, DR], F32, tag="sin", bufs=2)
            nc.sync.dma_start(out=c_t, in_=cos_d[_ts(st, 128), :])
            nc.sync.dma_start(out=s_t, in_=sin_d[_ts(st, 128), :])
            cos_sb.append(c_t)
            sin_sb.append(s_t)
            cq_t = small.tile([128, NH, DR], F32, tag="cosq", bufs=2)
            sq_t = small.tile([128, NH, DR], F32, tag="sinq", bufs=2)
            # broadcast over heads via stride-0 DMA read
            src_c = bass.AP(tensor=cos_d.tensor, offset=st * 128 * DR,
                            ap=[[DR, 128], [0, NH], [1, DR]])
            src_s = bass.AP(tensor=sin_d.tensor, offset=st * 128 * DR,
                            ap=[[DR, 128], [0, NH], [1, DR]])
            nc.sync.dma_start(out=cq_t, in_=src_c)
            nc.sync.dma_start(out=sq_t, in_=src_s)
            cosq.append(cq_t)
            sinq.append(sq_t)

        def rope_pair(out_ap, xe, xo, cos_half0, sin_half0, cos_half1, sin_half1,
                      scratch_pool, shape):
            """out[..0:h] = xe*c0 - xo*s0 ; out[..h:2h] = xo*c1 + xe*s1
            (APs pre-sliced; shape = scratch shape)."""
            h = shape[-1]
            t0 = scratch_pool.tile(shape, F32, tag="ropes0", bufs=2, name="t0")
            t1 = scratch_pool.tile(shape, F32, tag="ropes1", bufs=2, name="t1")
            o0, o1 = out_ap
            nc.vector.tensor_mul(t0, xe, cos_half0)
            nc.vector.tensor_mul(t1, xo, sin_half0)
            nc.vector.tensor_sub(o0, t0, t1)
            nc.vector.tensor_mul(t0, xo, cos_half1)
            nc.vector.tensor_mul(t1, xe, sin_half1)
            nc.vector.tensor_add(o1, t0, t1)

        # wq_b pool opens early; its DMAs are issued after the AllGather so
        # they don't queue ahead of the latency-critical ckv path
        wqb_stack = ExitStack()
        wqbp = wqb_stack.enter_context(tc.tile_pool(name="wqbp", bufs=1))
        wqb_sb = []

        # =========== phase 1: hidden^T, ckv, AllGather, cq ===========
        with tc.tile_pool(name="ph1", bufs=1) as ph1, \
             tc.tile_pool(name="ps1", bufs=2, space="PSUM") as ps1:

            # hidden rows -> SBUF, then PE-transpose to hsT tiles [128h, 256s]
            hs_sb, hsT = [], []
            for st in range(ST):
                h_t = ph1.tile([128, H], BF16, tag=f"hs{st}", name="h_t")
                nc.sync.dma_start(out=h_t, in_=hs_d[_ts(st, 128), :])
                hs_sb.append(h_t)
            for ht in range(H // 128):
                hT_t = ph1.tile([128, SPC], BF16, tag=f"hsT{ht}", name="hT_t")
                for st in range(ST):
                    p_t = ps1.tile([128, 128], BF16, tag="tp", name="p_t")
                    nc.tensor.transpose(out=p_t, in_=hs_sb[st][:, _ts(ht, 128)],
                                        identity=ident)
                    nc.scalar.copy(out=hT_t[:, _ts(st, 128)], in_=p_t)
                hsT.append(hT_t)

            # --- ckv = hidden @ wkv_a ; rmsnorm + rope; transpose -> ag_in
            wkva_sb = []
            for ht in range(H // 128):
                wk_t = ph1.tile([128, KVLR + DR], BF16, tag=f"wkva{ht}",
                                name="wk_t")
                nc.sync.dma_start(out=wk_t, in_=wkva_d[_ts(ht, 128), :])
                wkva_sb.append(wk_t)

            ckvn_bf, krot_bf = [], []
            for st in range(ST):
                ckv_p = ps1.tile([128, KVLR + DR], F32, tag="ckv", bufs=1,
                                 name="ckv_p")
                nh = H // 128
                for ht in range(nh):
                    nc.tensor.matmul(out=ckv_p[:, 0:KVLR],
                                     lhsT=hsT[ht][:, _ts(st, 128)],
                                     rhs=wkva_sb[ht][:, 0:KVLR],
                                     start=(ht == 0), stop=(ht == nh - 1))
                for ht in range(nh):
                    nc.tensor.matmul(out=ckv_p[:, KVLR:KVLR + DR],
                                     lhsT=hsT[ht][:, _ts(st, 128)],
                                     rhs=wkva_sb[ht][:, KVLR:KVLR + DR],
                                     start=(ht == 0), stop=(ht == nh - 1))
                # rmsnorm over KVLR (PSUM->SBUF copy first: DVE 2-input ops
                # may read at most one operand from PSUM)
                ckv_f = small.tile([128, KVLR], F32, tag="ckvf", bufs=2,
                                   name="ckv_f")
                nc.scalar.copy(out=ckv_f, in_=ckv_p[:, 0:KVLR])
                sq = small.tile([128, KVLR], F32, tag="sqscr", bufs=2, name="sq")
                ssq = small.tile([128, 1], F32, tag="ssq", name="ssq")
                nc.scalar.activation(out=sq, in_=ckv_f, func=AF.Square,
                                     accum_out=ssq)
                rstd = small.tile([128, 1], F32, tag="rstd", name="rstd")
                nc.scalar.activation(out=rstd, in_=ssq, func=AF.Sqrt,
                                     scale=1.0 / KVLR, bias=eps_t)
                nc.vector.reciprocal(out=rstd, in_=rstd)
                cn_t = ph1.tile([128, KVLR], BF16, tag=f"ckvn{st}", name="cn_t")
                nc.vector.tensor_scalar_mul(cn_t, ckv_f, rstd)
                ckvn_bf.append(cn_t)
                # rope on k_rot (fp32 from PSUM), out bf16
                kr_t = ph1.tile([128, DR], BF16, tag=f"krot{st}", name="kr_t")
                rope_pair(
                    (kr_t[:, 0:DR // 2], kr_t[:, DR // 2:DR]),
                    ckv_p[:, KVLR + 0:KVLR + DR:2], ckv_p[:, KVLR + 1:KVLR + DR:2],
                    cos_sb[st][:, 0:DR // 2], sin_sb[st][:, 0:DR // 2],
                    cos_sb[st][:, DR // 2:DR], sin_sb[st][:, DR // 2:DR],
                    small, [128, DR // 2])
                krot_bf.append(kr_t)

            # transpose [256, 576] -> X^T [576, 256] and stage to DRAM
            for kt in range(KVLR // 128):
                for st in range(ST):
                    p_t = ps1.tile([128, 128], BF16, tag="tp", name="p_t")
                    nc.tensor.transpose(out=p_t, in_=ckvn_bf[st][:, _ts(kt, 128)],
                                        identity=ident)
                    x_t = cps.tile([128, 128], BF16, tag="xT", name="x_t")
                    nc.scalar.copy(out=x_t, in_=p_t)
                    nc.sync.dma_start(
                        out=ag_in[_ts(kt, 128), _ts(st, 128)], in_=x_t)
            for st in range(ST):
                p_t = ps1.tile([64, 128], BF16, tag="tpr", bufs=1, name="p_t")
                nc.tensor.transpose(out=p_t, in_=krot_bf[st], identity=ident)
                x_t = cps.tile([64, 128], BF16, tag="xTr", name="x_t")
                nc.scalar.copy(out=x_t, in_=p_t)
                nc.sync.dma_start(out=ag_in[KVLR:KVLR + DR, _ts(st, 128)],
                                  in_=x_t)

            # ---- issue AllGather early ----
            collective("AllGather", ag_in, ag_out)

            # --- cq = hidden @ wq_a ; rmsnorm -> cqn (bf16) ---
            wqa_sb = []
            for ht in range(H // 128):
                wq_t = ph1.tile([128, QLR], BF16, tag=f"wqa{ht}", name="wq_t")
                nc.sync.dma_start(out=wq_t, in_=wqa_d[_ts(ht, 128), :])
                wqa_sb.append(wq_t)

            # first half of wq_b prefetch (rest when phase 2 opens)
            for rt in range(QLR // 256):
                wb_t = wqbp.tile([128, NH * DQK], BF16, tag=f"wqb{rt}",
                                 name="wb_t")
                nc.sync.dma_start(out=wb_t, in_=wqb_d[_ts(rt, 128), :])
                wqb_sb.append(wb_t)


            cqn_bf = []
            for st in range(ST):
                cq_f = ph1.tile([128, QLR], F32, tag=f"cqf{st}", name="cq_f")
                nh = H // 128
                # rb inner so each hsT stationary serves 3 matmuls (1 ldweights)
                cq_ps = [ps1.tile([128, 512], F32, tag="mm", bufs=3,
                                  name="cq_p") for _ in range(QLR // 512)]
                for ht in range(nh):
                    for rb in range(QLR // 512):
                        nc.tensor.matmul(out=cq_ps[rb],
                                         lhsT=hsT[ht][:, _ts(st, 128)],
                                         rhs=wqa_sb[ht][:, _ts(rb, 512)],
                                         start=(ht == 0), stop=(ht == nh - 1))
                for rb in range(QLR // 512):
                    nc.scalar.copy(out=cq_f[:, _ts(rb, 512)], in_=cq_ps[rb])
                sqq = ph1.tile([128, QLR], F32, tag="sqq", bufs=1, name="sqq")
                ssq = small.tile([128, 1], F32, tag="ssq", name="ssq")
                nc.scalar.activation(out=sqq, in_=cq_f, func=AF.Square,
                                     accum_out=ssq)
                rstd = small.tile([128, 1], F32, tag="rstd", name="rstd")
                nc.scalar.activation(out=rstd, in_=ssq, func=AF.Sqrt,
                                     scale=1.0 / QLR, bias=eps_t)
                nc.vector.reciprocal(out=rstd, in_=rstd)
                cn_t = ph1.tile([128, QLR], BF16, tag=f"cqn{st}", name="cn_t")
                nc.vector.tensor_scalar_mul(cn_t, cq_f, rstd)
                cqn_bf.append(cn_t)

            # transpose cqn -> cqnT [1536, 256] (interph: outlives phase 1)
            cqnT = []
            for rt in range(QLR // 128):
                cT_t = interph.tile([128, SPC], BF16, tag=f"cqnT{rt}",
                                    name="cT_t")
                for st in range(ST):
                    p_t = ps1.tile([128, 128], BF16, tag="tp", name="p_t")
                    nc.tensor.transpose(out=p_t,
                                        in_=cqn_bf[st][:, _ts(rt, 128)],
                                        identity=ident)
                    nc.scalar.copy(out=cT_t[:, _ts(st, 128)], in_=p_t)
                cqnT.append(cT_t)

        # =========== phase 2: q_b + rope -> AllToAll q ===========
        with tc.tile_pool(name="ph2", bufs=1) as ph2, \
             tc.tile_pool(name="ps2", bufs=2, space="PSUM") as ps2:
            nr = QLR // 128
            for rt in range(QLR // 256, nr):
                wb_t = ph2.tile([128, NH * DQK], BF16, tag=f"wqb{rt}",
                                name="wb_t")
                nc.sync.dma_start(out=wb_t, in_=wqb_d[_ts(rt, 128), :])
                wqb_sb.append(wb_t)
            for st in range(ST):
                q_f = ph2.tile([128, NH * DQK], F32, tag=f"qf{st}", name="q_f")
                # nb inner: each cqnT stationary serves 6 matmuls (1 ldweights)
                q_ps = [ps2.tile([128, 512], F32, tag="mm", bufs=6,
                                 name="q_p") for _ in range(NH * DQK // 512)]
                for rt in range(nr):
                    for nb in range(NH * DQK // 512):
                        nc.tensor.matmul(out=q_ps[nb],
                                         lhsT=cqnT[rt][:, _ts(st, 128)],
                                         rhs=wqb_sb[rt][:, _ts(nb, 512)],
                                         start=(rt == 0), stop=(rt == nr - 1))
                for nb in range(NH * DQK // 512):
                    nc.scalar.copy(out=q_f[:, _ts(nb, 512)], in_=q_ps[nb])
                # build bf16 q with rope applied to [:, h, DN:DQK]
                q_bf = ph2.tile([128, NH, DQK], BF16, tag=f"qbf{st}",
                                name="q_bf")
                qv = q_f.rearrange("p (h d) -> p h d", h=NH)
                nc.vector.tensor_copy(out=q_bf[:, :, 0:DN], in_=qv[:, :, 0:DN])
                hw = DR // 2
                rope_pair(
                    (q_bf[:, :, DN:DN + hw], q_bf[:, :, DN + hw:DQK]),
                    qv[:, :, DN + 0:DQK:2], qv[:, :, DN + 1:DQK:2],
                    cosq[st][:, :, 0:hw], sinq[st][:, :, 0:hw],
                    cosq[st][:, :, hw:DR], sinq[st][:, :, hw:DR],
                    small, [128, NH, hw])
                # stage per-destination chunks of the q AllToAll
                for j in range(NC):
                    nc.sync.dma_start(
                        out=a2aq_in[j * SPC + st * 128:j * SPC + (st + 1) * 128, :],
                        in_=q_bf[:, j * HPC:(j + 1) * HPC, :])
            collective("AllToAll", a2aq_in, a2aq_out)

        wqb_stack.close()  # free wq_b SBUF for wo

        # wo pool opens before phase 3 (LIFO); DMAs issued mid-attention
        wop = ctx.enter_context(tc.tile_pool(name="wop", bufs=1))
        wo_sb = []

        # =========== phase 3: k/v per head, qT, attention ===========
        with tc.tile_pool(name="ph3", bufs=1) as ph3, \
             tc.tile_pool(name="ph3b", bufs=4) as ph3b:
            s3 = ExitStack()
            ps3 = s3.enter_context(tc.tile_pool(name="ps3", bufs=2,
                                                space="PSUM"))

            wkvb_sb = []
            for kt in range(KVLR // 128):
                wv_t = ph3.tile([128, HPC * (DN + DV)], BF16, tag=f"wkvb{kt}",
                                name="wv_t")
                nc.sync.dma_start(out=wv_t, in_=wkvb_d[_ts(kt, 128), :])
                wkvb_sb.append(wv_t)

            # k_rot^T gathered: [64, 2048]
            krT = ph3.tile([64, S], BF16, tag="krT", name="krT")
            for g in range(NC):
                nc.sync.dma_start(
                    out=krT[:, _ts(g, SPC)],
                    in_=ag_out[g * (KVLR + DR) + KVLR:(g + 1) * (KVLR + DR), :])

            # k^T (nope) and v per head
            kT = [ph3.tile([128, S], BF16, tag=f"kT{h}", name="kT_t")
                  for h in range(HPC)]
            v_sb = [[ph3.tile([128, DV], BF16, tag=f"v{h}_{kc}", name="v_t")
                     for kc in range(S // 128)] for h in range(HPC)]
            nkt = KVLR // 128
            for g in range(NC):
                xk = []
                for kt in range(nkt):
                    xk_t = ph3b.tile([128, SPC], BF16, tag="xk", bufs=12,
                                     name="xk_t")
                    nc.sync.dma_start(
                        out=xk_t,
                        in_=ag_out[g * (KVLR + DR) + kt * 128:
                                   g * (KVLR + DR) + (kt + 1) * 128, :])
                    xk.append(xk_t)
                # k^T: wkvb stationary, one per (h, kt)
                kps = [ps3.tile([128, SPC], F32, tag="mmk", bufs=2, name="kp")
                       for _ in range(HPC)]
                for kt in range(nkt):
                    for h in range(HPC):
                        nc.tensor.matmul(
                            out=kps[h],
                            lhsT=wkvb_sb[kt][:, h * (DN + DV):h * (DN + DV) + DN],
                            rhs=xk[kt], start=(kt == 0), stop=(kt == nkt - 1))
                for h in range(HPC):
                    nc.scalar.copy(out=kT[h][:, _ts(g, SPC)], in_=kps[h])
                # v: xk stationary shared across both heads (1 ldweights / 2 mm)
                for sub in range(ST):
                    vps = [ps3.tile([128, DV], F32, tag="mmv", bufs=2,
                                    name="vp") for _ in range(HPC)]
                    for kt in range(nkt):
                        for h in range(HPC):
                            nc.tensor.matmul(
                                out=vps[h], lhsT=xk[kt][:, _ts(sub, 128)],
                                rhs=wkvb_sb[kt][:, h * (DN + DV) + DN:(h + 1) * (DN + DV)],
                                start=(kt == 0), stop=(kt == nkt - 1))
                    for h in range(HPC):
                        nc.vector.tensor_copy(out=v_sb[h][g * ST + sub],
                                              in_=vps[h])

            # qT per head: [128, 2048] nope + [64, 2048] rope
            qTn = [ph3.tile([128, S], BF16, tag=f"qTn{h}", name="qTn_t")
                   for h in range(HPC)]
            qTr = [ph3.tile([64, S], BF16, tag=f"qTr{h}", name="qTr_t")
                   for h in range(HPC)]
            for qt in range(S // 128):
                qblk = ph3b.tile([128, HPC * DQK], BF16, tag="qblk",
                                 name="qblk")
                nc.sync.dma_start(out=qblk, in_=a2aq_out[_ts(qt, 128), :])
                for h in range(HPC):
                    p_t = ps3.tile([128, 128], BF16, tag="tp", name="p_t")
                    nc.tensor.transpose(out=p_t,
                                        in_=qblk[:, h * DQK:h * DQK + DN],
                                        identity=ident)
                    nc.scalar.copy(out=qTn[h][:, _ts(qt, 128)], in_=p_t)
                    pr_t = ps3.tile([64, 128], BF16, tag="tpr", name="pr_t")
                    nc.tensor.transpose(out=pr_t,
                                        in_=qblk[:, h * DQK + DN:(h + 1) * DQK],
                                        identity=ident)
                    nc.scalar.copy(out=qTr[h][:, _ts(qt, 128)], in_=pr_t)

            # wo loads overlap the attention phase (issued after the
            # latency-critical kv/qT DMAs)
            for gt in range(NH * DV // 128):
                wo_t = wop.tile([128, H], BF16, tag=f"wo{gt}", name="wo_t")
                nc.sync.dma_start(out=wo_t, in_=wo_d[_ts(gt, 128), :])
                wo_sb.append(wo_t)

            # ---- attention (heads interleaved; scores in <=1024 halves) ----
            s3.close()  # release ps3's PSUM banks before the attention pools
            attnT = [ph3.tile([DV, S], BF16, tag=f"attnT{h}", name="attnT_t")
                     for h in range(HPC)]
            with tc.tile_pool(name="pssc", bufs=1, space="PSUM") as pssc, \
                 tc.tile_pool(name="pspv", bufs=2, space="PSUM") as pspv:
                for qt in range(S // 128):
                    for h in range(HPC):
                        ki = (qt + 1) * 128
                        probs = ph3b.tile([128, S], BF16, tag=f"probs{h}",
                                          bufs=2, name="probs")
                        sumes = []
                        for half in range((ki + 1023) // 1024):
                            h0, h1 = half * 1024, min(ki, (half + 1) * 1024)
                            sc_p = pssc.tile([128, 1024], F32, tag=f"sc{h}",
                                             name="sc_p")
                            for kb in range((h1 - h0 + 511) // 512):
                                k0 = h0 + kb * 512
                                k1 = min(h1, k0 + 512)
                                nc.tensor.matmul(out=sc_p[:, k0 - h0:k1 - h0],
                                                 lhsT=qTn[h][:, _ts(qt, 128)],
                                                 rhs=kT[h][:, k0:k1],
                                                 start=True, stop=False)
                                nc.tensor.matmul(out=sc_p[:, k0 - h0:k1 - h0],
                                                 lhsT=qTr[h][:, _ts(qt, 128)],
                                                 rhs=krT[:, k0:k1],
                                                 start=False, stop=True)
                            if h1 == ki:
                                # causal mask on the diagonal block
                                nc.vector.tensor_add(
                                    sc_p[:, ki - 128 - h0:ki - h0],
                                    sc_p[:, ki - 128 - h0:ki - h0], cmask)
                            sume = small.tile([128, 1], F32, tag="sume",
                                              bufs=8, name="sume")
                            nc.scalar.activation(out=probs[:, h0:h1],
                                                 in_=sc_p[:, 0:h1 - h0],
                                                 func=AF.Exp, accum_out=sume)
                            sumes.append(sume)
                        # softmax denom (no max subtraction: logits are O(1))
                        while len(sumes) > 1:
                            s_new = small.tile([128, 1], F32, tag="sume",
                                               bufs=8, name="s_new")
                            nc.vector.tensor_add(s_new, sumes[0], sumes[1])
                            sumes = [s_new] + sumes[2:]
                        rec = small.tile([128, 1], F32, tag="rec", bufs=4,
                                         name="rec")
                        nc.vector.reciprocal(out=rec, in_=sumes[0])
                        nc.vector.tensor_scalar_mul(probs[:, 0:ki],
                                                    probs[:, 0:ki], rec)
                        # transpose probs chunks; PV accumulates attn^T
                        pv_p = pspv.tile([DV, 128], F32, tag=f"pv{h}",
                                         bufs=1, name="pv_p")
                        for kc in range(qt + 1):
                            pt_p = pspv.tile([128, 128], BF16, tag="ptp",
                                             name="pt_p")
                            nc.tensor.transpose(
                                out=pt_p, in_=probs[:, _ts(kc, 128)],
                                identity=ident)
                            pT_sb = ph3b.tile([128, 128], BF16, tag="pT",
                                              bufs=4, name="pT_sb")
                            nc.vector.tensor_copy(out=pT_sb, in_=pt_p)
                            nc.tensor.matmul(out=pv_p, lhsT=v_sb[h][kc],
                                             rhs=pT_sb, start=(kc == 0),
                                             stop=(kc == qt))
                        nc.scalar.copy(out=attnT[h][:, _ts(qt, 128)],
                                       in_=pv_p)

            # attention-output AllToAll (head-shard -> row-shard)
            a2ao_in = nc.dram_tensor("a2ao_in", [NH * DV, SPC], BF16).ap()
            a2ao_out = nc.dram_tensor("a2ao_out", [NH * DV, SPC], BF16).ap()
            for j in range(NC):
                for h in range(HPC):
                    nc.sync.dma_start(
                        out=a2ao_in[j * HPC * DV + h * DV:
                                    j * HPC * DV + (h + 1) * DV, :],
                        in_=attnT[h][:, _ts(j, SPC)])
            collective("AllToAll", a2ao_in, a2ao_out)

        # =========== phase 4: o_proj on row shard ===========
        with tc.tile_pool(name="ph4", bufs=1) as ph4, \
             tc.tile_pool(name="ps4", bufs=3, space="PSUM") as ps4:
            wo_sb = []
            for gt in range(NH * DV // 128):
                wo_t = ph4.tile([128, H], BF16, tag=f"wo{gt}", name="wo_t")
                nc.sync.dma_start(out=wo_t, in_=wo_d[_ts(gt, 128), :])
                wo_sb.append(wo_t)
            at_sb = []
            for gt in range(NH * DV // 128):
                a_t = ph4.tile([128, SPC], BF16, tag=f"at{gt}", name="a_t")
                nc.sync.dma_start(out=a_t, in_=a2ao_out[_ts(gt, 128), :])
                at_sb.append(a_t)
            ngt = NH * DV // 128
            # head-0 chunks (even gt) first: their AllToAll lands earlier;
            # hb inner so each at_sb stationary serves 4 matmuls
            gt_order = list(range(ngt))
            for st in range(ST):
                o_ps = [ps4.tile([128, 512], F32, tag="mm", bufs=4,
                                 name="o_p") for _ in range(H // 512)]
                for i, gt in enumerate(gt_order):
                    for hb in range(H // 512):
                        nc.tensor.matmul(out=o_ps[hb],
                                         lhsT=at_sb[gt][:, _ts(st, 128)],
                                         rhs=wo_sb[gt][:, _ts(hb, 512)],
                                         start=(i == 0), stop=(i == ngt - 1))
                for hb in range(H // 512):
                    o_t = cps.tile([128, 512], F32, tag="osb", name="o_t")
                    nc.scalar.copy(out=o_t, in_=o_ps[hb])
                    nc.sync.dma_start(out=out_d[_ts(st, 128), _ts(hb, 512)],
                                      in_=o_t)

    nc.compile()
    return nc


def _prep(hidden_states, cos, sin, wq_a, q_ln, wq_b, wkv_a, kv_ln, wkv_b, wo):
    """Host-side sharding + weight prep (fold layernorm weights + softmax
    scale into the B projections, cast to bf16)."""
    bf = ml_dtypes.bfloat16
    hs = hidden_states.reshape(S, H)
    cos2 = np.ascontiguousarray(cos.reshape(S, DR).astype(np.float32))
    sin2 = np.ascontiguousarray(sin.reshape(S, DR).astype(np.float32))
    wqa = wq_a.astype(bf)
    wkva = wkv_a.astype(bf)
    scale = np.float32(DQK) ** np.float32(-0.5)
    wqb = (wq_b * q_ln[:, None] * scale).astype(bf)
    wkvb = (wkv_b * kv_ln[:, None]).astype(bf)
    wob = wo.astype(bf)

    in_maps = []
    for c in range(NC):
        r = slice(c * SPC, (c + 1) * SPC)
        hcols = slice(c * HPC * (DN + DV), (c + 1) * HPC * (DN + DV))
        in_maps.append({
            "hs": np.ascontiguousarray(hs[r].astype(bf)),
            "cosr": np.ascontiguousarray(cos2[r]),
            "sinr": np.ascontiguousarray(sin2[r]),
            "wqa": wqa,
            "wkva": wkva,
            "wqb": wqb,
            "wkvb": np.ascontiguousarray(wkvb[:, hcols]),
            "wo": wob,
        })
    return in_maps


def kernel(**inputs) -> np.ndarray:
    if "nc" not in _CACHED:
        _CACHED["nc"] = build()
    nc = _CACHED["nc"]
    in_maps = _prep(**inputs)
    res = run_bass_kernel_spmd(nc, in_maps, list(range(NC)))
    out = np.concatenate([res.results[c]["out"] for c in range(NC)], axis=0)
    return out.reshape(B, S, H).astype(np.float32)


if __name__ == "__main__":
    rng = np.random.RandomState(0)
    ins = {
        "hidden_states": rng.randn(B, S, H).astype(np.float32),
        "cos": rng.rand(B, S, DR).astype(np.float32),
        "sin": rng.rand(B, S, DR).astype(np.float32),
        "wq_a": (rng.randn(H, QLR) * 0.02).astype(np.float32),
        "q_ln": np.ones(QLR, np.float32),
        "wq_b": (rng.randn(QLR, NH * DQK) * 0.02).astype(np.float32),
        "wkv_a": (rng.randn(H, KVLR + DR) * 0.02).astype(np.float32),
        "kv_ln": np.ones(KVLR, np.float32),
        "wkv_b": (rng.randn(KVLR, NH * (DN + DV)) * 0.02).astype(np.float32),
        "wo": (rng.randn(NH * DV, H) * 0.02).astype(np.float32),
    }
    out = kernel(**ins)
    print("kernel out", out.shape, out.dtype, np.abs(out).mean())



# revision 10
# speedup vs baseline: 1.1772x; 1.1772x over previous
"""DeepseekV32 MLA-style attention on 8 Trainium2 NeuronCores (Bass/Tile).

v2 design:
- Row shard (256 rows/core) for the low-rank A projections and q_b; head
  shard (2 heads/core) for kv_b expansion + attention + o_proj.
- Host prep: hidden is pre-transposed (hsT input), rope columns of wkv_a /
  wq_b are pre-permuted so the de-interleave is free, wo is sliced per-core
  by head, layernorm weights and softmax scale are folded into the B
  projections.
- Exactly two collectives (they serialize on the collective engine):
  AllGather of X^T=[ckv_normed; roped k_rot] and AllToAll of q
  (row-shard -> head-shard).  The output projection is computed per-head
  (partial over all rows) and the 8 partials are summed on the host, which
  removes the output collective entirely.
- Attention computes scores TRANSPOSED (k on partitions, q on free dim):
  probsT = exp(scoresT) feeds the PV matmul directly (no transposes, no
  PSUM->SBUF probs copies).  The softmax denominator comes from a
  ones-vector matmul accumulated alongside PV; normalization is applied
  while draining attT via partition_broadcast of the reciprocal.

All matmuls run in bf16 with fp32 PSUM accumulation; softmax and rmsnorm
statistics are fp32.
"""
import sys

sys.path.insert(0, "/opt/trn_rl_repo")

import numpy as np
import ml_dtypes
from contextlib import ExitStack

import concourse.bass as bass
import concourse.tile as tile
import concourse.mybir as mybir
from concourse import bacc
from concourse.masks import make_identity
from concourse.bass_utils import run_bass_kernel_spmd

BF16 = mybir.dt.bfloat16
F32 = mybir.dt.float32
AF = mybir.ActivationFunctionType

NC = 8            # cores
B, S, H = 1, 2048, 2048
NH = 16           # heads
QLR = 1536        # q lora rank
KVLR = 512        # kv lora rank
DR = 64           # rope dim
DN = 128          # nope dim
DV = 128          # v dim
DQK = DN + DR     # 192
EPS = 1e-6
HPC = NH // NC    # heads per core = 2
SPC = S // NC     # seq rows per core = 256
ST = SPC // 128   # row tiles per core = 2
NEG = -1e30       # causal mask fill
NKT = KVLR // 128  # 4

_CACHED = {}


def _ts(i, n):
    return slice(i * n, (i + 1) * n)


def build():
    nc = bacc.Bacc("TRN2", target_bir_lowering=False, debug=False,
                   num_devices=NC)

    # ---- kernel I/O (per-core shards / replicated weights) ----
    hsT_d = nc.dram_tensor("hsT", [H, SPC], BF16, kind="ExternalInput").ap()
    cos_d = nc.dram_tensor("cosr", [SPC, DR], F32, kind="ExternalInput").ap()
    sin_d = nc.dram_tensor("sinr", [SPC, DR], F32, kind="ExternalInput").ap()
    wqa_d = nc.dram_tensor("wqa", [H, QLR], BF16, kind="ExternalInput").ap()
    wkva_d = nc.dram_tensor("wkva", [H, KVLR + DR], BF16,
                            kind="ExternalInput").ap()
    wqb_d = nc.dram_tensor("wqb", [QLR, NH * DQK], BF16,
                           kind="ExternalInput").ap()
    wkvb_d = nc.dram_tensor("wkvb", [KVLR, HPC * (DN + DV)], BF16,
                            kind="ExternalInput").ap()
    wo_d = nc.dram_tensor("wo", [HPC * DV, H], BF16, kind="ExternalInput").ap()
    # per-core output: partial o_proj (this core's 2 heads) over ALL rows
    out_d = nc.dram_tensor("out", [S, H], BF16, kind="ExternalOutput").ap()

    # ---- collective buffers ----
    # X = [ckv_normed^T (512) ; roped k_rot^T (64)] per core, gathered over S
    ag_in = nc.dram_tensor("ag_in", [KVLR + DR, SPC], BF16).ap()
    ag_out = nc.dram_tensor("ag_out", [NC * (KVLR + DR), SPC], BF16,
                            addr_space="Shared").ap()
    # q: row-shard [256, all 16 heads] -> head-shard [2048, 2 heads]
    a2aq_in = nc.dram_tensor("a2aq_in", [S, HPC * DQK], BF16).ap()
    a2aq_out = nc.dram_tensor("a2aq_out", [S, HPC * DQK], BF16).ap()

    rg = [list(range(NC))]

    with tile.TileContext(nc) as tc, ExitStack() as ctx:
        singles = ctx.enter_context(tc.tile_pool(name="singles", bufs=1))
        small = ctx.enter_context(tc.tile_pool(name="small", bufs=4))

        ident = singles.tile([128, 128], BF16)
        make_identity(nc, ident)
        eps_t = singles.tile([128, 1], F32)
        nc.vector.memset(eps_t, float(EPS))
        ones_bf = singles.tile([128, 1], BF16)
        nc.vector.memset(ones_bf, 1.0)
        # causal additive mask for a diagonal 128x128 block of scoresT:
        # cmaskT[k, q] = 0 if k <= q else NEG
        cmaskT = singles.tile([128, 128], F32)
        nc.gpsimd.memset(cmaskT, 0.0)
        nc.gpsimd.affine_select(
            out=cmaskT, in_=cmaskT, compare_op=mybir.AluOpType.is_ge,
            fill=NEG, base=0, pattern=[[1, 128]], channel_multiplier=-1)

        # =========== phase 1: ckv -> X^T -> AllGather ===========
        # (wq_b pool opens lazily below; wo pool persists to the end)
        hsT_sb = []
        wkva_sb = []
        wop = ctx.enter_context(tc.tile_pool(name="wop", bufs=1))
        interph = ctx.enter_context(tc.tile_pool(name="interph", bufs=1))

        with tc.tile_pool(name="ph1", bufs=1) as ph1, \
             tc.tile_pool(name="ps1", bufs=2, space="PSUM") as ps1:
            # interleaved weight/activation loads pace the ckv matmuls
            for ht in range(H // 128):
                wk_t = ph1.tile([128, KVLR + DR], BF16, tag=f"wkva{ht}",
                                name="wk_t")
                nc.sync.dma_start(out=wk_t, in_=wkva_d[_ts(ht, 128), :])
                wkva_sb.append(wk_t)
                h_t = interph.tile([128, SPC], BF16, tag=f"hsT{ht}",
                                   name="h_t")
                nc.sync.dma_start(out=h_t, in_=hsT_d[_ts(ht, 128), :])
                hsT_sb.append(h_t)

            cos_sb, sin_sb = [], []
            for st in range(ST):
                c_t = small.tile([128, DR], F32, tag="cos", bufs=2)
                s_t = small.tile([128, DR], F32, tag="sin", bufs=2)
                nc.sync.dma_start(out=c_t, in_=cos_d[_ts(st, 128), :])
                nc.sync.dma_start(out=s_t, in_=sin_d[_ts(st, 128), :])
                cos_sb.append(c_t)
                sin_sb.append(s_t)

            # ckv matmuls: ht outer so each weight tile is consumed on
            # arrival; both row-tiles accumulate in parallel PSUM banks
            ckv_ps = [ps1.tile([128, KVLR], F32, tag=f"ckv{st}", bufs=1,
                               name="ckv_p") for st in range(ST)]
            rope_ps = [ps1.tile([128, DR], F32, tag=f"ckr{st}", bufs=1,
                                name="rope_p") for st in range(ST)]
            nh = H // 128
            for ht in range(nh):
                for st in range(ST):
                    nc.tensor.matmul(out=ckv_ps[st],
                                     lhsT=hsT_sb[ht][:, _ts(st, 128)],
                                     rhs=wkva_sb[ht][:, 0:KVLR],
                                     start=(ht == 0), stop=(ht == nh - 1))
            for ht in range(nh):
                for st in range(ST):
                    nc.tensor.matmul(out=rope_ps[st],
                                     lhsT=hsT_sb[ht][:, _ts(st, 128)],
                                     rhs=wkva_sb[ht][:, KVLR:KVLR + DR],
                                     start=(ht == 0), stop=(ht == nh - 1))

            hw = DR // 2
            for st in range(ST):
                # rmsnorm over KVLR (copy out of PSUM first: DVE 2-input ops
                # may read at most one operand from PSUM)
                ckv_f = small.tile([128, KVLR], F32, tag="ckvf", bufs=2,
                                   name="ckv_f")
                nc.scalar.copy(out=ckv_f, in_=ckv_ps[st])
                sq = small.tile([128, KVLR], F32, tag="sqscr", bufs=2,
                                name="sq")
                ssq = small.tile([128, 1], F32, tag="ssq", name="ssq")
                nc.scalar.activation(out=sq, in_=ckv_f, func=AF.Square,
                                     accum_out=ssq)
                rstd = small.tile([128, 1], F32, tag="rstd", name="rstd")
                nc.scalar.activation(out=rstd, in_=ssq, func=AF.Sqrt,
                                     scale=1.0 / KVLR, bias=eps_t)
                nc.vector.reciprocal(out=rstd, in_=rstd)
                cn_t = ph1.tile([128, KVLR], BF16, tag=f"ckvn{st}",
                                name="cn_t")
                nc.vector.tensor_scalar_mul(cn_t, ckv_f, rstd)
                # rope on k_rot (weights pre-permuted -> contiguous halves)
                kr_t = ph1.tile([128, DR], BF16, tag=f"krot{st}", name="kr_t")
                t0 = small.tile([128, hw], F32, tag="krs0", bufs=2, name="t0")
                t1 = small.tile([128, hw], F32, tag="krs1", bufs=2, name="t1")
                xe, xo = rope_ps[st][:, 0:hw], rope_ps[st][:, hw:DR]
                nc.vector.tensor_mul(t0, xe, cos_sb[st][:, 0:hw])
                nc.vector.tensor_mul(t1, xo, sin_sb[st][:, 0:hw])
                nc.vector.tensor_sub(kr_t[:, 0:hw], t0, t1)
                nc.vector.tensor_mul(t0, xo, cos_sb[st][:, hw:DR])
                nc.vector.tensor_mul(t1, xe, sin_sb[st][:, hw:DR])
                nc.vector.tensor_add(kr_t[:, hw:DR], t0, t1)

                # transpose [128, 512+64] -> X^T columns, stage to DRAM
                for kt in range(NKT):
                    p_t = ps1.tile([128, 128], BF16, tag="tp", name="p_t")
                    nc.tensor.transpose(out=p_t, in_=cn_t[:, _ts(kt, 128)],
                                        identity=ident)
                    x_t = small.tile([128, 128], BF16, tag="xT", name="x_t")
                    nc.scalar.copy(out=x_t, in_=p_t)
                    nc.sync.dma_start(out=ag_in[_ts(kt, 128), _ts(st, 128)],
                                      in_=x_t)
                pr_t = ps1.tile([64, 128], BF16, tag="tpr", name="pr_t")
                nc.tensor.transpose(out=pr_t, in_=kr_t, identity=ident)
                xr_t = small.tile([64, 128], BF16, tag="xTr", name="xr_t")
                nc.scalar.copy(out=xr_t, in_=pr_t)
                nc.sync.dma_start(out=ag_in[KVLR:KVLR + DR, _ts(st, 128)],
                                  in_=xr_t)

            # ---- collective 1: AllGather X^T ----
            nc.gpsimd.collective_compute(
                "AllGather", mybir.AluOpType.bypass, replica_groups=rg,
                ins=[ag_in.opt()], outs=[ag_out.opt()])

        # =========== phase 2: cq -> q_b -> rope -> AllToAll q ===========
        wqb_stack = ExitStack()
        wqbp = wqb_stack.enter_context(tc.tile_pool(name="wqbp", bufs=1))
        wkvb_sb = []
        wo_sb = []
        with tc.tile_pool(name="ph2", bufs=1) as ph2:
            wqa_sb = []
            for ht in range(H // 128):
                wq_t = ph2.tile([128, QLR], BF16, tag=f"wqa{ht}", name="wq_t")
                nc.sync.dma_start(out=wq_t, in_=wqa_d[_ts(ht, 128), :])
                wqa_sb.append(wq_t)
            # small weights next in queue: kv_b (needed right after AG)
            for kt in range(NKT):
                wv_t = interph.tile([128, HPC * (DN + DV)], BF16,
                                    tag=f"wkvb{kt}", name="wv_t")
                nc.sync.dma_start(out=wv_t, in_=wkvb_d[_ts(kt, 128), :])
                wkvb_sb.append(wv_t)
            wqb_sb = []
            for rt in range(QLR // 128):
                wb_t = wqbp.tile([128, NH * DQK], BF16, tag=f"wqb{rt}",
                                 name="wb_t")
                nc.sync.dma_start(out=wb_t, in_=wqb_d[_ts(rt, 128), :])
                wqb_sb.append(wb_t)
            for gt in range(HPC):
                wo_t = wop.tile([128, H], BF16, tag=f"wo{gt}", name="wo_t")
                nc.sync.dma_start(out=wo_t, in_=wo_d[_ts(gt, 128), :])
                wo_sb.append(wo_t)

            # cq: ht outer (stream wqa), both row-tiles in parallel.
            # cq PSUM banks (6) are scoped so they free before q_b's (8).
            cqstack = ExitStack()
            ps2a = cqstack.enter_context(tc.tile_pool(name="ps2a", bufs=1,
                                                      space="PSUM"))
            cq_ps = [[ps2a.tile([128, 512], F32, tag=f"cq{st}_{rb}", bufs=1,
                                name="cq_p") for rb in range(QLR // 512)]
                     for st in range(ST)]
            for ht in range(nh):
                for st in range(ST):
                    for rb in range(QLR // 512):
                        nc.tensor.matmul(out=cq_ps[st][rb],
                                         lhsT=hsT_sb[ht][:, _ts(st, 128)],
                                         rhs=wqa_sb[ht][:, _ts(rb, 512)],
                                         start=(ht == 0), stop=(ht == nh - 1))
            cqn_bf = []
            for st in range(ST):
                cq_f = ph2.tile([128, QLR], F32, tag=f"cqf{st}", name="cq_f")
                for rb in range(QLR // 512):
                    nc.scalar.copy(out=cq_f[:, _ts(rb, 512)],
                                   in_=cq_ps[st][rb])
                sqq = ph2.tile([128, QLR], F32, tag="sqq", bufs=1, name="sqq")
                ssq = small.tile([128, 1], F32, tag="ssq", name="ssq")
                nc.scalar.activation(out=sqq, in_=cq_f, func=AF.Square,
                                     accum_out=ssq)
                rstd = small.tile([128, 1], F32, tag="rstd", name="rstd")
                nc.scalar.activation(out=rstd, in_=ssq, func=AF.Sqrt,
                                     scale=1.0 / QLR, bias=eps_t)
                nc.vector.reciprocal(out=rstd, in_=rstd)
                cn_t = ph2.tile([128, QLR], BF16, tag=f"cqn{st}", name="cn_t")
                nc.vector.tensor_scalar_mul(cn_t, cq_f, rstd)
                cqn_bf.append(cn_t)

            # transpose cqn -> cqnT [1536, 256]
            cqnT = []
            for rt in range(QLR // 128):
                cT_t = ph2.tile([128, SPC], BF16, tag=f"cqnT{rt}",
                                name="cT_t")
                for st in range(ST):
                    p_t = ps2a.tile([128, 128], BF16, tag="tp", bufs=2,
                                    name="p_t")
                    nc.tensor.transpose(out=p_t,
                                        in_=cqn_bf[st][:, _ts(rt, 128)],
                                        identity=ident)
                    nc.scalar.copy(out=cT_t[:, _ts(st, 128)], in_=p_t)
                cqnT.append(cT_t)
            cqstack.close()

            # q_b per row-tile; psum in head-pair blocks of 384 cols so the
            # rope slicing never crosses a PSUM tile boundary
            s2b = ExitStack()
            ps2b = s2b.enter_context(tc.tile_pool(name="ps2b", bufs=1,
                                                  space="PSUM"))
            nr = QLR // 128
            hw = DR // 2
            for st in range(ST):
                q_ps = [ps2b.tile([128, HPC * DQK], F32, tag=f"qb{nb}",
                                  bufs=1, name="q_p") for nb in range(NC)]
                for rt in range(nr):
                    for nb in range(NC):
                        nc.tensor.matmul(out=q_ps[nb],
                                         lhsT=cqnT[rt][:, _ts(st, 128)],
                                         rhs=wqb_sb[rt][:, _ts(nb, HPC * DQK)],
                                         start=(rt == 0), stop=(rt == nr - 1))
                # rope + bf16 pack, one head-pair psum tile at a time
                q_bf = ph2.tile([128, NH, DQK], BF16, tag=f"qbf{st}",
                                name="q_bf")
                for nb in range(NC):
                    qv = q_ps[nb].rearrange("p (h d) -> p h d", h=HPC)
                    dst = q_bf[:, nb * HPC:(nb + 1) * HPC, :]
                    nc.vector.tensor_copy(out=dst[:, :, 0:DN],
                                          in_=qv[:, :, 0:DN])
                    # cos/sin broadcast over the head axis via stride-0 AP
                    def _bc(t, lo, hi):
                        return bass.AP(
                            tensor=t.tensor, offset=t.offset + lo,
                            ap=[list(t.ap[0]), [0, HPC], [1, hi - lo]])
                    cs, sn = cos_sb[st], sin_sb[st]
                    xe, xo = qv[:, :, DN:DN + hw], qv[:, :, DN + hw:DQK]
                    t0 = small.tile([128, HPC, hw], F32, tag="qrs0", bufs=2,
                                    name="t0")
                    t1 = small.tile([128, HPC, hw], F32, tag="qrs1", bufs=2,
                                    name="t1")
                    nc.vector.tensor_mul(t0, xe, _bc(cs, 0, hw))
                    nc.vector.tensor_mul(t1, xo, _bc(sn, 0, hw))
                    nc.vector.tensor_sub(dst[:, :, DN:DN + hw], t0, t1)
                    nc.vector.tensor_mul(t0, xo, _bc(cs, hw, DR))
                    nc.vector.tensor_mul(t1, xe, _bc(sn, hw, DR))
                    nc.vector.tensor_add(dst[:, :, DN + hw:DQK], t0, t1)
                # stage per-destination chunks of the q AllToAll
                for j in range(NC):
                    nc.sync.dma_start(
                        out=a2aq_in[j * SPC + st * 128:
                                    j * SPC + (st + 1) * 128, :],
                        in_=q_bf[:, j * HPC:(j + 1) * HPC, :])
            s2b.close()
            # ---- collective 2: AllToAll q ----
            nc.gpsimd.collective_compute(
                "AllToAll", mybir.AluOpType.bypass, replica_groups=rg,
                ins=[a2aq_in.opt()], outs=[a2aq_out.opt()])
        wqb_stack.close()

        # =========== phase 3: k/v expansion per head ===========
        with tc.tile_pool(name="ph3", bufs=1) as ph3, \
             tc.tile_pool(name="ph3b", bufs=4) as ph3b:
            s3 = ExitStack()
            ps3 = s3.enter_context(tc.tile_pool(name="ps3", bufs=2,
                                                space="PSUM"))
            krT = ph3.tile([64, S], BF16, tag="krT", name="krT")
            for g in range(NC):
                nc.sync.dma_start(
                    out=krT[:, _ts(g, SPC)],
                    in_=ag_out[g * (KVLR + DR) + KVLR:
                               (g + 1) * (KVLR + DR), :])
            kT = [ph3.tile([128, S], BF16, tag=f"kT{h}", name="kT_t")
                  for h in range(HPC)]
            v_sb = [[ph3.tile([128, DV], BF16, tag=f"v{h}_{kc}", name="v_t")
                     for kc in range(S // 128)] for h in range(HPC)]
            for g in range(NC):
                xk = []
                for kt in range(NKT):
                    xk_t = ph3b.tile([128, SPC], BF16, tag="xk", bufs=12,
                                     name="xk_t")
                    nc.sync.dma_start(
                        out=xk_t,
                        in_=ag_out[g * (KVLR + DR) + kt * 128:
                                   g * (KVLR + DR) + (kt + 1) * 128, :])
                    xk.append(xk_t)
                kps = [ps3.tile([128, SPC], F32, tag="mmk", bufs=2, name="kp")
                       for _ in range(HPC)]
                for kt in range(NKT):
                    for h in range(HPC):
                        nc.tensor.matmul(
                            out=kps[h],
                            lhsT=wkvb_sb[kt][:, h * (DN + DV):
                                             h * (DN + DV) + DN],
                            rhs=xk[kt], start=(kt == 0), stop=(kt == NKT - 1))
                for h in range(HPC):
                    nc.scalar.copy(out=kT[h][:, _ts(g, SPC)], in_=kps[h])
                for sub in range(ST):
                    vps = [ps3.tile([128, DV], F32, tag="mmv", bufs=2,
                                    name="vp") for _ in range(HPC)]
                    for kt in range(NKT):
                        for h in range(HPC):
                            nc.tensor.matmul(
                                out=vps[h], lhsT=xk[kt][:, _ts(sub, 128)],
                                rhs=wkvb_sb[kt][:, h * (DN + DV) + DN:
                                                (h + 1) * (DN + DV)],
                                start=(kt == 0), stop=(kt == NKT - 1))
                    for h in range(HPC):
                        nc.vector.tensor_copy(out=v_sb[h][g * ST + sub],
                                              in_=vps[h])

            # ---- q^T per head from the AllToAll ----
            qTn = [ph3.tile([128, S], BF16, tag=f"qTn{h}", name="qTn_t")
                   for h in range(HPC)]
            qTr = [ph3.tile([64, S], BF16, tag=f"qTr{h}", name="qTr_t")
                   for h in range(HPC)]
            for qt in range(S // 128):
                qblk = ph3b.tile([128, HPC * DQK], BF16, tag="qblk",
                                 name="qblk")
                nc.sync.dma_start(out=qblk, in_=a2aq_out[_ts(qt, 128), :])
                for h in range(HPC):
                    p_t = ps3.tile([128, 128], BF16, tag="tp", name="p_t")
                    nc.tensor.transpose(out=p_t,
                                        in_=qblk[:, h * DQK:h * DQK + DN],
                                        identity=ident)
                    if h == 0:
                        nc.scalar.copy(out=qTn[h][:, _ts(qt, 128)], in_=p_t)
                    else:
                        nc.vector.tensor_copy(out=qTn[h][:, _ts(qt, 128)],
                                              in_=p_t)
                    pr_t = ps3.tile([64, 128], BF16, tag="tpr", name="pr_t")
                    nc.tensor.transpose(out=pr_t,
                                        in_=qblk[:, h * DQK + DN:
                                                 (h + 1) * DQK],
                                        identity=ident)
                    if h == 0:
                        nc.scalar.copy(out=qTr[h][:, _ts(qt, 128)], in_=pr_t)
                    else:
                        nc.vector.tensor_copy(out=qTr[h][:, _ts(qt, 128)],
                                              in_=pr_t)
            s3.close()

            # =========== phase 4: attention (scoresT) ===========
            QB = 512          # q columns per block
            NQB = S // QB     # 4
            attTn = [[None] * NQB for _ in range(HPC)]
            with tc.tile_pool(name="ps5", bufs=1, space="PSUM") as ps5:
                for qb in range(NQB):
                    for h in range(HPC):
                        attp = ps5.tile([128, QB], F32, tag="attT", bufs=2,
                                        name="attp")
                        denp = ps5.tile([1, QB], F32, tag="den", bufs=2,
                                        name="denp")
                        nkc = 4 * qb + 4
                        for kc in range(nkc):
                            off = max(0, (kc - 4 * qb) * 128)
                            scp = ps5.tile([128, QB], F32, tag="scT", bufs=2,
                                           name="scp")
                            nc.tensor.matmul(
                                out=scp[:, off:QB],
                                lhsT=kT[h][:, _ts(kc, 128)],
                                rhs=qTn[h][:, qb * QB + off:(qb + 1) * QB],
                                start=True, stop=False)
                            nc.tensor.matmul(
                                out=scp[:, off:QB],
                                lhsT=krT[:, _ts(kc, 128)],
                                rhs=qTr[h][:, qb * QB + off:(qb + 1) * QB],
                                start=False, stop=True)
                            if kc >= 4 * qb:
                                nc.vector.tensor_add(scp[:, off:off + 128],
                                                     scp[:, off:off + 128],
                                                     cmaskT)
                            probsT = ph3b.tile([128, QB], BF16, tag="probsT",
                                               bufs=4, name="probsT")
                            if off > 0:
                                nc.vector.memset(probsT[:, 0:off], 0.0)
                            nc.scalar.activation(out=probsT[:, off:QB],
                                                 in_=scp[:, off:QB],
                                                 func=AF.Exp)
                            nc.tensor.matmul(out=attp, lhsT=v_sb[h][kc],
                                             rhs=probsT,
                                             start=(kc == 0),
                                             stop=(kc == nkc - 1))
                            nc.tensor.matmul(out=denp, lhsT=ones_bf,
                                             rhs=probsT,
                                             start=(kc == 0),
                                             stop=(kc == nkc - 1))
                        # normalize while draining attT
                        rec = small.tile([1, QB], F32, tag="rec", bufs=4,
                                         name="rec")
                        nc.vector.reciprocal(out=rec, in_=denp)
                        bca = small.tile([128, QB], F32, tag="bca", bufs=2,
                                         name="bca")
                        nc.gpsimd.partition_broadcast(bca, rec)
                        a_t = ph3.tile([128, QB], BF16, tag=f"attn{h}_{qb}",
                                       name="a_t")
                        nc.vector.tensor_mul(a_t, attp, bca)
                        attTn[h][qb] = a_t

            # =========== phase 5: partial o_proj (all rows, 2 heads) ======
            with tc.tile_pool(name="ps6", bufs=1, space="PSUM") as ps6, \
                 tc.tile_pool(name="ph6", bufs=6) as ph6:
                for qs in range(S // 128):
                    qb, sub = qs // 4, qs % 4
                    for cb in range(H // 512):
                        op = ps6.tile([128, 512], F32, tag="op", bufs=6,
                                      name="op")
                        for h in range(HPC):
                            nc.tensor.matmul(
                                out=op,
                                lhsT=attTn[h][qb][:, _ts(sub, 128)],
                                rhs=wo_sb[h][:, _ts(cb, 512)],
                                start=(h == 0), stop=(h == HPC - 1))
                        o_t = ph6.tile([128, 512], BF16, tag="osb",
                                       name="o_t")
                        if (qs + cb) % 2 == 0:
                            nc.scalar.copy(out=o_t, in_=op)
                        else:
                            nc.vector.tensor_copy(out=o_t, in_=op)
                        nc.sync.dma_start(
                            out=out_d[_ts(qs, 128), _ts(cb, 512)],
                            in_=o_t)

    nc.compile()
    return nc


def _prep(hidden_states, cos, sin, wq_a, q_ln, wq_b, wkv_a, kv_ln, wkv_b, wo):
    """Host-side sharding + weight prep: pre-transpose hidden, fold layernorm
    weights + softmax scale into the B projections, pre-permute rope columns
    (de-interleave), slice wo by head, cast to bf16."""
    bf = ml_dtypes.bfloat16
    hsT = np.ascontiguousarray(hidden_states.reshape(S, H).T.astype(bf))
    cos2 = np.ascontiguousarray(cos.reshape(S, DR).astype(np.float32))
    sin2 = np.ascontiguousarray(sin.reshape(S, DR).astype(np.float32))

    # de-interleave permutation for a 64-wide rope slice
    perm = np.concatenate([np.arange(0, DR, 2), np.arange(1, DR, 2)])

    wkva = np.array(wkv_a, copy=True)
    wkva[:, KVLR:] = wkva[:, KVLR:][:, perm]
    wkva = wkva.astype(bf)

    scale = np.float32(DQK) ** np.float32(-0.5)
    wqb = np.asarray(wq_b * q_ln[:, None] * scale)
    wqb = wqb.reshape(QLR, NH, DQK)
    wqb = np.concatenate([wqb[:, :, :DN], wqb[:, :, DN:][:, :, perm]],
                         axis=2).reshape(QLR, NH * DQK).astype(bf)

    wkvb = (wkv_b * kv_ln[:, None]).astype(bf)
    wob = wo.astype(bf)

    in_maps = []
    for c in range(NC):
        r = slice(c * SPC, (c + 1) * SPC)
        hcols = slice(c * HPC * (DN + DV), (c + 1) * HPC * (DN + DV))
        hrows = slice(c * HPC * DV, (c + 1) * HPC * DV)
        in_maps.append({
            "hsT": np.ascontiguousarray(hsT[:, r]),
            "cosr": np.ascontiguousarray(cos2[r]),
            "sinr": np.ascontiguousarray(sin2[r]),
            "wqa": wq_a.astype(bf),
            "wkva": wkva,
            "wqb": wqb,
            "wkvb": np.ascontiguousarray(wkvb[:, hcols]),
            "wo": np.ascontiguousarray(wob[hrows]),
        })
    return in_maps


def kernel(**inputs) -> np.ndarray:
    if "nc" not in _CACHED:
        _CACHED["nc"] = build()
    nc = _CACHED["nc"]
    in_maps = _prep(**inputs)
    res = run_bass_kernel_spmd(nc, in_maps, list(range(NC)))
    out = np.zeros((S, H), np.float32)
    for c in range(NC):
        out += res.results[c]["out"].astype(np.float32)
    return out.reshape(B, S, H)


if __name__ == "__main__":
    rng = np.random.RandomState(0)
    ins = {
        "hidden_states": rng.randn(B, S, H).astype(np.float32),
        "cos": rng.rand(B, S, DR).astype(np.float32),
        "sin": rng.rand(B, S, DR).astype(np.float32),
        "wq_a": (rng.randn(H, QLR) * 0.02).astype(np.float32),
        "q_ln": np.ones(QLR, np.float32),
        "wq_b": (rng.randn(QLR, NH * DQK) * 0.02).astype(np.float32),
        "wkv_a": (rng.randn(H, KVLR + DR) * 0.02).astype(np.float32),
        "kv_ln": np.ones(KVLR, np.float32),
        "wkv_b": (rng.randn(KVLR, NH * (DN + DV)) * 0.02).astype(np.float32),
        "wo": (rng.randn(NH * DV, H) * 0.02).astype(np.float32),
    }
    out = kernel(**ins)
    print("kernel out", out.shape, out.dtype, np.abs(out).mean())


# revision 11
# speedup vs baseline: 1.3243x; 1.1249x over previous
"""DeepseekV32 MLA-style attention on 8 Trainium2 NeuronCores (Bass/Tile).

Sharding: row shard (256 rows/core) for the low-rank A projections and q_b;
head shard (2 heads/core) for kv_b expansion + attention + o_proj.  Host
prep: hidden is pre-transposed (hsT input), rope columns of wkv_a / wq_b are
pre-permuted so the de-interleave is free, wo is sliced per-core by head,
layernorm weights and softmax scale are folded into the B projections.

Exactly two collectives (they serialize on the collective engine): AllGather
of X^T=[ckv_normed; roped k_rot] and AllToAll of q (row-shard ->
head-shard).  The output projection is computed per-head (a partial over all
rows) and the 8 partials are summed on the host, which removes the output
collective entirely.

Attention computes scores TRANSPOSED (k on partitions, q on free dim):
probsT = exp(scoresT) feeds the PV matmul directly (no transposes, no
PSUM->SBUF probs copies).  The softmax denominator comes from a ones-vector
matmul accumulated alongside PV; normalization is applied while draining
attT via partition_broadcast of the reciprocal.  o_proj is interleaved into
the attention stream as each 512-column block of both heads completes.

All matmuls run in bf16 with fp32 PSUM accumulation; softmax and rmsnorm
statistics are fp32.
"""
import sys

sys.path.insert(0, "/opt/trn_rl_repo")

import numpy as np
import ml_dtypes
from contextlib import ExitStack

import concourse.bass as bass
import concourse.tile as tile
import concourse.mybir as mybir
from concourse import bacc
from concourse.masks import make_identity
from concourse.bass_utils import run_bass_kernel_spmd

BF16 = mybir.dt.bfloat16
F32 = mybir.dt.float32
AF = mybir.ActivationFunctionType

NC = 8            # cores
B, S, H = 1, 2048, 2048
NH = 16           # heads
QLR = 1536        # q lora rank
KVLR = 512        # kv lora rank
DR = 64           # rope dim
DN = 128          # nope dim
DV = 128          # v dim
DQK = DN + DR     # 192
EPS = 1e-6
HPC = NH // NC    # heads per core = 2
SPC = S // NC     # seq rows per core = 256
ST = SPC // 128   # row tiles per core = 2
NEG = -1e30       # causal mask fill
NKT = KVLR // 128  # 4
HW = DR // 2      # 32

_CACHED = {}


def _ts(i, n):
    return slice(i * n, (i + 1) * n)


def _chunked(dram_ap, nchunk, rows, cols):
    """AP reading `nchunk` consecutive [rows, cols] row-blocks of a 2-D dram
    tensor as one [rows, nchunk, cols] transfer."""
    return bass.AP(tensor=dram_ap.tensor, offset=0,
                   ap=[[cols, rows], [rows * cols, nchunk], [1, cols]])


def build():
    nc = bacc.Bacc("TRN2", target_bir_lowering=False, debug=False,
                   num_devices=NC)

    # ---- kernel I/O (per-core shards / replicated weights) ----
    hsT_d = nc.dram_tensor("hsT", [H, SPC], BF16, kind="ExternalInput").ap()
    cos_d = nc.dram_tensor("cosr", [SPC, DR], F32, kind="ExternalInput").ap()
    sin_d = nc.dram_tensor("sinr", [SPC, DR], F32, kind="ExternalInput").ap()
    wqa_d = nc.dram_tensor("wqa", [H, QLR], BF16, kind="ExternalInput").ap()
    wkva_d = nc.dram_tensor("wkva", [H, KVLR + DR], BF16,
                            kind="ExternalInput").ap()
    wqb_d = nc.dram_tensor("wqb", [QLR, NH * DQK], BF16,
                           kind="ExternalInput").ap()
    wkvb_d = nc.dram_tensor("wkvb", [KVLR, HPC * (DN + DV)], BF16,
                            kind="ExternalInput").ap()
    wo_d = nc.dram_tensor("wo", [HPC * DV, H], BF16, kind="ExternalInput").ap()
    # per-core output: partial o_proj (this core's 2 heads) over ALL rows
    out_d = nc.dram_tensor("out", [S, H], BF16, kind="ExternalOutput").ap()

    # ---- collective buffers ----
    ag_in = nc.dram_tensor("ag_in", [KVLR + DR, SPC], BF16).ap()
    ag_out = nc.dram_tensor("ag_out", [NC * (KVLR + DR), SPC], BF16,
                            addr_space="Shared").ap()
    a2aq_in = nc.dram_tensor("a2aq_in", [S, HPC * DQK], BF16).ap()
    a2aq_out = nc.dram_tensor("a2aq_out", [S, HPC * DQK], BF16).ap()

    rg = [list(range(NC))]
    nh = H // 128

    with tile.TileContext(nc) as tc, ExitStack() as ctx:
        singles = ctx.enter_context(tc.tile_pool(name="singles", bufs=1))
        small = ctx.enter_context(tc.tile_pool(name="small", bufs=4))

        ident = singles.tile([128, 128], BF16)
        make_identity(nc, ident)
        eps_t = singles.tile([128, 1], F32)
        nc.vector.memset(eps_t, float(EPS))
        ones_bf = singles.tile([128, 1], BF16)
        nc.vector.memset(ones_bf, 1.0)
        # cmaskT[k, q] = 0 if k <= q else NEG (diagonal block of scoresT)
        cmaskT = singles.tile([128, 128], F32)
        nc.gpsimd.memset(cmaskT, 0.0)
        nc.gpsimd.affine_select(
            out=cmaskT, in_=cmaskT, compare_op=mybir.AluOpType.is_ge,
            fill=NEG, base=0, pattern=[[1, 128]], channel_multiplier=-1)

        hsT_sb = []
        wkva_sb = []
        wop = ctx.enter_context(tc.tile_pool(name="wop", bufs=1))
        interph = ctx.enter_context(tc.tile_pool(name="interph", bufs=1))

        # =========== phase 1: ckv -> X^T -> AllGather ===========
        with tc.tile_pool(name="ph1", bufs=1) as ph1, \
             tc.tile_pool(name="ps1", bufs=2, space="PSUM") as ps1:
            # batched input DMAs (one transfer per tensor; the SP queue's
            # per-DMA dispatch cost would otherwise pace the whole phase)
            wkva_all = ph1.tile([128, nh, KVLR + DR], BF16, tag="wkva",
                                name="wkva_all")
            nc.sync.dma_start(out=wkva_all,
                              in_=_chunked(wkva_d, nh, 128, KVLR + DR))
            wkva_sb = [wkva_all[:, ht, :] for ht in range(nh)]
            hsT_all = interph.tile([128, nh, SPC], BF16, tag="hsT",
                                   name="hsT_all")
            nc.sync.dma_start(out=hsT_all, in_=_chunked(hsT_d, nh, 128, SPC))
            hsT_sb = [hsT_all[:, ht, :] for ht in range(nh)]

            cos_all = singles.tile([128, ST, DR], F32, name="cos_all")
            sin_all = singles.tile([128, ST, DR], F32, name="sin_all")
            nc.sync.dma_start(out=cos_all, in_=_chunked(cos_d, ST, 128, DR))
            nc.sync.dma_start(out=sin_all, in_=_chunked(sin_d, ST, 128, DR))
            cos_sb = [cos_all[:, st, :] for st in range(ST)]
            sin_sb = [sin_all[:, st, :] for st in range(ST)]

            ckv_ps = [ps1.tile([128, KVLR], F32, tag=f"ckv{st}", bufs=1,
                               name="ckv_p") for st in range(ST)]
            rope_ps = [ps1.tile([128, DR], F32, tag=f"ckr{st}", bufs=1,
                                name="rope_p") for st in range(ST)]
            for ht in range(nh):
                for st in range(ST):
                    nc.tensor.matmul(out=ckv_ps[st],
                                     lhsT=hsT_sb[ht][:, _ts(st, 128)],
                                     rhs=wkva_sb[ht][:, 0:KVLR],
                                     start=(ht == 0), stop=(ht == nh - 1))
            for ht in range(nh):
                for st in range(ST):
                    nc.tensor.matmul(out=rope_ps[st],
                                     lhsT=hsT_sb[ht][:, _ts(st, 128)],
                                     rhs=wkva_sb[ht][:, KVLR:KVLR + DR],
                                     start=(ht == 0), stop=(ht == nh - 1))

            for st in range(ST):
                # rmsnorm over KVLR, stats straight off PSUM
                sq = small.tile([128, KVLR], F32, tag="sqscr", bufs=2,
                                name="sq")
                ssq = small.tile([128, 1], F32, tag="ssq", name="ssq")
                nc.scalar.activation(out=sq, in_=ckv_ps[st], func=AF.Square,
                                     accum_out=ssq)
                rstd = small.tile([128, 1], F32, tag="rstd", name="rstd")
                nc.scalar.activation(out=rstd, in_=ssq, func=AF.Sqrt,
                                     scale=1.0 / KVLR, bias=eps_t)
                nc.vector.reciprocal(out=rstd, in_=rstd)
                cn_t = ph1.tile([128, KVLR], BF16, tag=f"ckvn{st}",
                                name="cn_t")
                nc.vector.tensor_scalar_mul(cn_t, ckv_ps[st], rstd)
                # rope on k_rot (weights pre-permuted -> contiguous halves)
                kr_t = ph1.tile([128, DR], BF16, tag=f"krot{st}", name="kr_t")
                t0 = small.tile([128, HW], F32, tag="krs0", bufs=2, name="t0")
                t1 = small.tile([128, HW], F32, tag="krs1", bufs=2, name="t1")
                xe, xo = rope_ps[st][:, 0:HW], rope_ps[st][:, HW:DR]
                nc.vector.tensor_mul(t0, xe, cos_sb[st][:, 0:HW])
                nc.vector.tensor_mul(t1, xo, sin_sb[st][:, 0:HW])
                nc.vector.tensor_sub(kr_t[:, 0:HW], t0, t1)
                nc.vector.tensor_mul(t0, xo, cos_sb[st][:, HW:DR])
                nc.vector.tensor_mul(t1, xe, sin_sb[st][:, HW:DR])
                nc.vector.tensor_add(kr_t[:, HW:DR], t0, t1)

                # transpose [128, 512+64] -> X^T, stage with 2 DMAs per st
                xp = ps1.tile([128, KVLR], BF16, tag="xp", name="xp")
                for kt in range(NKT):
                    nc.tensor.transpose(out=xp[:, _ts(kt, 128)],
                                        in_=cn_t[:, _ts(kt, 128)],
                                        identity=ident)
                x_t = small.tile([128, NKT, 128], BF16, tag="xT", bufs=2,
                                 name="x_t")
                if st == 0:
                    nc.vector.tensor_copy(out=x_t,
                                          in_=xp.rearrange(
                                              "p (k c) -> p k c", k=NKT))
                else:
                    nc.scalar.copy(out=x_t,
                                   in_=xp.rearrange("p (k c) -> p k c",
                                                    k=NKT))
                dst = bass.AP(tensor=ag_in.tensor, offset=st * 128,
                              ap=[[SPC, 128], [128 * SPC, NKT], [1, 128]])
                nc.sync.dma_start(out=dst, in_=x_t)
                pr_t = ps1.tile([64, 128], BF16, tag="tpr", name="pr_t")
                nc.tensor.transpose(out=pr_t, in_=kr_t, identity=ident)
                xr_t = small.tile([64, 128], BF16, tag="xTr", name="xr_t")
                nc.vector.tensor_copy(out=xr_t, in_=pr_t)
                nc.sync.dma_start(out=ag_in[KVLR:KVLR + DR, _ts(st, 128)],
                                  in_=xr_t)

            # ---- collective 1: AllGather X^T ----
            nc.gpsimd.collective_compute(
                "AllGather", mybir.AluOpType.bypass, replica_groups=rg,
                ins=[ag_in.opt()], outs=[ag_out.opt()])

        # =========== phase 2: cq -> q_b -> rope -> AllToAll q ===========
        wqb_stack = ExitStack()
        wqbp = wqb_stack.enter_context(tc.tile_pool(name="wqbp", bufs=1))
        wkvb_sb = []
        wo_sb = []
        with tc.tile_pool(name="ph2", bufs=1) as ph2:
            # wqa in 4 chunks so cq overlaps the transfer
            wqa_sb = []
            for cg in range(4):
                wq_t = ph2.tile([128, 4, QLR], BF16, tag=f"wqa{cg}",
                                name="wq_t")
                src = bass.AP(tensor=wqa_d.tensor, offset=cg * 4 * 128 * QLR,
                              ap=[[QLR, 128], [128 * QLR, 4], [1, QLR]])
                nc.sync.dma_start(out=wq_t, in_=src)
                wqa_sb += [wq_t[:, i, :] for i in range(4)]
            wkvb_all = interph.tile([128, NKT, HPC * (DN + DV)], BF16,
                                    tag="wkvb", name="wkvb_all")
            nc.sync.dma_start(out=wkvb_all,
                              in_=_chunked(wkvb_d, NKT, 128,
                                           HPC * (DN + DV)))
            wkvb_sb = [wkvb_all[:, kt, :] for kt in range(NKT)]
            wqb_sb = []
            for cg in range(4):
                wb_t = wqbp.tile([128, 3, NH * DQK], BF16, tag=f"wqb{cg}",
                                 name="wb_t")
                src = bass.AP(tensor=wqb_d.tensor,
                              offset=cg * 3 * 128 * NH * DQK,
                              ap=[[NH * DQK, 128], [128 * NH * DQK, 3],
                                  [1, NH * DQK]])
                nc.sync.dma_start(out=wb_t, in_=src)
                wqb_sb += [wb_t[:, i, :] for i in range(3)]
            wo_all = wop.tile([128, HPC, H], BF16, tag="wo", name="wo_all")
            nc.sync.dma_start(out=wo_all, in_=_chunked(wo_d, HPC, 128, H))
            wo_sb = [wo_all[:, h, :] for h in range(HPC)]

            # cq: ht outer (stream wqa), both row-tiles in parallel
            s2a = ExitStack()
            ps2a = s2a.enter_context(tc.tile_pool(name="ps2a", bufs=1,
                                                  space="PSUM"))
            cq_ps = [[ps2a.tile([128, 512], F32, tag=f"cq{st}_{rb}", bufs=1,
                                name="cq_p") for rb in range(QLR // 512)]
                     for st in range(ST)]
            for ht in range(nh):
                for st in range(ST):
                    for rb in range(QLR // 512):
                        nc.tensor.matmul(out=cq_ps[st][rb],
                                         lhsT=hsT_sb[ht][:, _ts(st, 128)],
                                         rhs=wqa_sb[ht][:, _ts(rb, 512)],
                                         start=(ht == 0), stop=(ht == nh - 1))
            cqn_bf = []
            for st in range(ST):
                ssqs = []
                for rb in range(QLR // 512):
                    sqq = small.tile([128, 512], F32, tag="sqq", bufs=2,
                                     name="sqq")
                    ssq = small.tile([128, 1], F32, tag="ssq3", bufs=6,
                                     name="ssq")
                    nc.scalar.activation(out=sqq, in_=cq_ps[st][rb],
                                         func=AF.Square, accum_out=ssq)
                    ssqs.append(ssq)
                nc.vector.tensor_add(ssqs[0], ssqs[0], ssqs[1])
                nc.vector.tensor_add(ssqs[0], ssqs[0], ssqs[2])
                rstd = small.tile([128, 1], F32, tag="rstd", name="rstd")
                nc.scalar.activation(out=rstd, in_=ssqs[0], func=AF.Sqrt,
                                     scale=1.0 / QLR, bias=eps_t)
                nc.vector.reciprocal(out=rstd, in_=rstd)
                cn_t = ph2.tile([128, QLR], BF16, tag=f"cqn{st}", name="cn_t")
                for rb in range(QLR // 512):
                    nc.vector.tensor_scalar_mul(cn_t[:, _ts(rb, 512)],
                                                cq_ps[st][rb], rstd)
                cqn_bf.append(cn_t)

            # transpose cqn -> cqnT [1536, 256] (batched drains, alternating
            # engines)
            cqnT = []
            for rt in range(QLR // 128):
                cT_t = ph2.tile([128, SPC], BF16, tag=f"cqnT{rt}",
                                name="cT_t")
                p_t = ps2a.tile([128, SPC], BF16, tag="tp", bufs=2,
                                name="p_t")
                for st in range(ST):
                    nc.tensor.transpose(out=p_t[:, _ts(st, 128)],
                                        in_=cqn_bf[st][:, _ts(rt, 128)],
                                        identity=ident)
                if rt % 2 == 0:
                    nc.scalar.copy(out=cT_t, in_=p_t)
                else:
                    nc.vector.tensor_copy(out=cT_t, in_=p_t)
                cqnT.append(cT_t)
            s2a.close()

            # q_b per row-tile; psum in head-pair blocks of 384 cols so the
            # rope slicing never crosses a PSUM tile boundary
            s2b = ExitStack()
            ps2b = s2b.enter_context(tc.tile_pool(name="ps2b", bufs=1,
                                                  space="PSUM"))
            nr = QLR // 128
            for st in range(ST):
                q_ps = [ps2b.tile([128, HPC * DQK], F32, tag=f"qb{nb}",
                                  bufs=1, name="q_p") for nb in range(NC)]
                for rt in range(nr):
                    for nb in range(NC):
                        nc.tensor.matmul(out=q_ps[nb],
                                         lhsT=cqnT[rt][:, _ts(st, 128)],
                                         rhs=wqb_sb[rt][:, _ts(nb, HPC * DQK)],
                                         start=(rt == 0), stop=(rt == nr - 1))
                # rope + bf16 pack: nope copies on Act, rope muls on DVE
                q_bf = ph2.tile([128, NH, DQK], BF16, tag=f"qbf{st}",
                                name="q_bf")
                for nb in range(NC):
                    qv = q_ps[nb].rearrange("p (h d) -> p h d", h=HPC)
                    dst = q_bf[:, nb * HPC:(nb + 1) * HPC, :]
                    nc.scalar.copy(out=dst[:, :, 0:DN], in_=qv[:, :, 0:DN])

                    def _bc(t, lo, hi):
                        return bass.AP(
                            tensor=t.tensor, offset=t.offset + lo,
                            ap=[list(t.ap[0]), [0, HPC], [1, hi - lo]])
                    cs, sn = cos_sb[st], sin_sb[st]
                    xe, xo = qv[:, :, DN:DN + HW], qv[:, :, DN + HW:DQK]
                    t0 = small.tile([128, HPC, HW], F32, tag="qrs0", bufs=2,
                                    name="t0")
                    t1 = small.tile([128, HPC, HW], F32, tag="qrs1", bufs=2,
                                    name="t1")
                    nc.vector.tensor_mul(t0, xe, _bc(cs, 0, HW))
                    nc.vector.tensor_mul(t1, xo, _bc(sn, 0, HW))
                    nc.vector.tensor_sub(dst[:, :, DN:DN + HW], t0, t1)
                    nc.vector.tensor_mul(t0, xo, _bc(cs, HW, DR))
                    nc.vector.tensor_mul(t1, xe, _bc(sn, HW, DR))
                    nc.vector.tensor_add(dst[:, :, DN + HW:DQK], t0, t1)
                # stage the whole row-tile with one DMA (8 dest chunks)
                dst = bass.AP(tensor=a2aq_in.tensor,
                              offset=st * 128 * HPC * DQK,
                              ap=[[HPC * DQK, 128], [SPC * HPC * DQK, NC],
                                  [1, HPC * DQK]])
                nc.sync.dma_start(out=dst, in_=q_bf)
            s2b.close()
            # ---- collective 2: AllToAll q ----
            nc.gpsimd.collective_compute(
                "AllToAll", mybir.AluOpType.bypass, replica_groups=rg,
                ins=[a2aq_in.opt()], outs=[a2aq_out.opt()])
        wqb_stack.close()

        # =========== phase 3: k/v expansion + qT ===========
        with tc.tile_pool(name="ph3", bufs=1) as ph3, \
             tc.tile_pool(name="ph3b", bufs=4) as ph3b:
            s3 = ExitStack()
            ps3 = s3.enter_context(tc.tile_pool(name="ps3", bufs=2,
                                                space="PSUM"))
            krT = ph3.tile([64, NC, SPC], BF16, tag="krT", name="krT")
            src = bass.AP(tensor=ag_out.tensor, offset=KVLR * SPC,
                          ap=[[SPC, 64], [(KVLR + DR) * SPC, NC], [1, SPC]])
            nc.sync.dma_start(out=krT, in_=src)
            krTf = krT.rearrange("p g c -> p (g c)")

            kT = [ph3.tile([128, S], BF16, tag=f"kT{h}", name="kT_t")
                  for h in range(HPC)]
            v_sb = [[ph3.tile([128, DV], BF16, tag=f"v{h}_{kc}", name="v_t")
                     for kc in range(S // 128)] for h in range(HPC)]
            for g in range(NC):
                xk_t = ph3b.tile([128, NKT, SPC], BF16, tag="xk", bufs=3,
                                 name="xk_t")
                src = bass.AP(tensor=ag_out.tensor,
                              offset=g * (KVLR + DR) * SPC,
                              ap=[[SPC, 128], [128 * SPC, NKT], [1, SPC]])
                nc.sync.dma_start(out=xk_t, in_=src)
                xk = [xk_t[:, kt, :] for kt in range(NKT)]
                kps = [ps3.tile([128, SPC], F32, tag="mmk", bufs=2, name="kp")
                       for _ in range(HPC)]
                for kt in range(NKT):
                    for h in range(HPC):
                        nc.tensor.matmul(
                            out=kps[h],
                            lhsT=wkvb_sb[kt][:, h * (DN + DV):
                                             h * (DN + DV) + DN],
                            rhs=xk[kt], start=(kt == 0), stop=(kt == NKT - 1))
                for h in range(HPC):
                    if h == 0:
                        nc.scalar.copy(out=kT[h][:, _ts(g, SPC)], in_=kps[h])
                    else:
                        nc.vector.tensor_copy(out=kT[h][:, _ts(g, SPC)],
                                              in_=kps[h])
                for sub in range(ST):
                    vps = [ps3.tile([128, DV], F32, tag="mmv", bufs=2,
                                    name="vp") for _ in range(HPC)]
                    for kt in range(NKT):
                        for h in range(HPC):
                            nc.tensor.matmul(
                                out=vps[h], lhsT=xk[kt][:, _ts(sub, 128)],
                                rhs=wkvb_sb[kt][:, h * (DN + DV) + DN:
                                                (h + 1) * (DN + DV)],
                                start=(kt == 0), stop=(kt == NKT - 1))
                    for h in range(HPC):
                        nc.vector.tensor_copy(out=v_sb[h][g * ST + sub],
                                              in_=vps[h])

            # q^T per head from the AllToAll (batched loads + drains)
            qTn = [ph3.tile([128, S], BF16, tag=f"qTn{h}", name="qTn_t")
                   for h in range(HPC)]
            qTr = [ph3.tile([64, S], BF16, tag=f"qTr{h}", name="qTr_t")
                   for h in range(HPC)]
            for q4 in range(4):
                qblk = ph3b.tile([128, 4, HPC * DQK], BF16, tag="qblk",
                                 bufs=2, name="qblk")
                src = bass.AP(tensor=a2aq_out.tensor,
                              offset=q4 * 4 * 128 * HPC * DQK,
                              ap=[[HPC * DQK, 128], [128 * HPC * DQK, 4],
                                  [1, HPC * DQK]])
                nc.sync.dma_start(out=qblk, in_=src)
                for h in range(HPC):
                    pn = ps3.tile([128, 512], BF16, tag="tqn", bufs=2,
                                  name="pn")
                    pr = ps3.tile([64, 512], BF16, tag="tqr", bufs=2,
                                  name="pr")
                    for i in range(4):
                        nc.tensor.transpose(
                            out=pn[:, _ts(i, 128)],
                            in_=qblk[:, i, h * DQK:h * DQK + DN],
                            identity=ident)
                        nc.tensor.transpose(
                            out=pr[:, _ts(i, 128)],
                            in_=qblk[:, i, h * DQK + DN:(h + 1) * DQK],
                            identity=ident)
                    if h == 0:
                        nc.scalar.copy(out=qTn[h][:, _ts(q4, 512)], in_=pn)
                        nc.vector.tensor_copy(out=qTr[h][:, _ts(q4, 512)],
                                              in_=pr)
                    else:
                        nc.vector.tensor_copy(out=qTn[h][:, _ts(q4, 512)],
                                              in_=pn)
                        nc.scalar.copy(out=qTr[h][:, _ts(q4, 512)], in_=pr)
            s3.close()

            # ====== phase 4: attention (scoresT) + interleaved o_proj ======
            QB = 512
            NQB = S // QB
            attTn = [[None] * NQB for _ in range(HPC)]

            def oproj(qb, ps_pool, o_pool):
                """o_proj for q rows [qb*512, (qb+1)*512): both heads."""
                for sub in range(4):
                    qs = qb * 4 + sub
                    o_t = o_pool.tile([128, H], BF16, tag="osb", bufs=3,
                                      name="o_t")
                    for cb in range(H // 512):
                        op = ps_pool.tile([128, 512], F32, tag="op", bufs=2,
                                          name="op")
                        for h in range(HPC):
                            nc.tensor.matmul(
                                out=op,
                                lhsT=attTn[h][qb][:, _ts(sub, 128)],
                                rhs=wo_sb[h][:, _ts(cb, 512)],
                                start=(h == 0), stop=(h == HPC - 1))
                        if cb % 2 == 0:
                            nc.scalar.copy(out=o_t[:, _ts(cb, 512)], in_=op)
                        else:
                            nc.vector.tensor_copy(out=o_t[:, _ts(cb, 512)],
                                                  in_=op)
                    nc.sync.dma_start(out=out_d[_ts(qs, 128), :], in_=o_t)

            with tc.tile_pool(name="ps5", bufs=1, space="PSUM") as ps5, \
                 tc.tile_pool(name="ph5", bufs=1) as ph5:
                for qb in range(NQB):
                    for h in range(HPC):
                        attp = ps5.tile([128, QB], F32, tag="attT", bufs=2,
                                        name="attp")
                        denp = ps5.tile([1, QB], F32, tag="den", bufs=2,
                                        name="denp")
                        nkc = 4 * qb + 4
                        # software-pipelined: PV/den of kc trail the score
                        # matmuls of kc+1 so PE never waits on exp
                        probs = [None] * nkc

                        def scores(kc):
                            off = max(0, (kc - 4 * qb) * 128)
                            scp = ps5.tile([128, QB], F32, tag="scT", bufs=2,
                                           name="scp")
                            nc.tensor.matmul(
                                out=scp[:, off:QB],
                                lhsT=kT[h][:, _ts(kc, 128)],
                                rhs=qTn[h][:, qb * QB + off:(qb + 1) * QB],
                                start=True, stop=False)
                            nc.tensor.matmul(
                                out=scp[:, off:QB],
                                lhsT=krTf[:, _ts(kc, 128)],
                                rhs=qTr[h][:, qb * QB + off:(qb + 1) * QB],
                                start=False, stop=True)
                            if kc >= 4 * qb:
                                nc.vector.tensor_add(scp[:, off:off + 128],
                                                     scp[:, off:off + 128],
                                                     cmaskT)
                            pt = ph3b.tile([128, QB], BF16, tag="probsT",
                                           bufs=4, name="probsT")
                            if off > 0:
                                nc.vector.memset(pt[:, 0:off], 0.0)
                            nc.scalar.activation(out=pt[:, off:QB],
                                                 in_=scp[:, off:QB],
                                                 func=AF.Exp)
                            probs[kc] = pt

                        def pv(kc):
                            nc.tensor.matmul(out=attp, lhsT=v_sb[h][kc],
                                             rhs=probs[kc],
                                             start=(kc == 0),
                                             stop=(kc == nkc - 1))
                            nc.tensor.matmul(out=denp, lhsT=ones_bf,
                                             rhs=probs[kc],
                                             start=(kc == 0),
                                             stop=(kc == nkc - 1))

                        scores(0)
                        for kc in range(1, nkc):
                            scores(kc)
                            pv(kc - 1)
                        pv(nkc - 1)

                        # normalize while draining attT
                        rec = small.tile([1, QB], F32, tag="rec", bufs=4,
                                         name="rec")
                        nc.vector.reciprocal(out=rec, in_=denp)
                        bca = small.tile([128, QB], F32, tag="bca", bufs=2,
                                         name="bca")
                        nc.gpsimd.partition_broadcast(bca, rec)
                        a_t = ph5.tile([128, QB], BF16, tag=f"attn{h}_{qb}",
                                       name="a_t")
                        nc.vector.tensor_mul(a_t, attp, bca)
                        attTn[h][qb] = a_t
                    # both heads of qb are drained: stream its o_proj
                    oproj(qb, ps5, ph5)

    nc.compile()
    return nc


def _prep(hidden_states, cos, sin, wq_a, q_ln, wq_b, wkv_a, kv_ln, wkv_b, wo):
    """Host-side sharding + weight prep: pre-transpose hidden, fold layernorm
    weights + softmax scale into the B projections, pre-permute rope columns
    (de-interleave), slice wo by head, cast to bf16."""
    bf = ml_dtypes.bfloat16
    hsT = np.ascontiguousarray(hidden_states.reshape(S, H).T.astype(bf))
    cos2 = np.ascontiguousarray(cos.reshape(S, DR).astype(np.float32))
    sin2 = np.ascontiguousarray(sin.reshape(S, DR).astype(np.float32))

    # de-interleave permutation for a 64-wide rope slice
    perm = np.concatenate([np.arange(0, DR, 2), np.arange(1, DR, 2)])

    wkva = np.array(wkv_a, copy=True)
    wkva[:, KVLR:] = wkva[:, KVLR:][:, perm]
    wkva = wkva.astype(bf)

    scale = np.float32(DQK) ** np.float32(-0.5)
    wqb = np.asarray(wq_b * q_ln[:, None] * scale)
    wqb = wqb.reshape(QLR, NH, DQK)
    wqb = np.concatenate([wqb[:, :, :DN], wqb[:, :, DN:][:, :, perm]],
                         axis=2).reshape(QLR, NH * DQK).astype(bf)

    wkvb = (wkv_b * kv_ln[:, None]).astype(bf)
    wob = wo.astype(bf)

    in_maps = []
    for c in range(NC):
        r = slice(c * SPC, (c + 1) * SPC)
        hcols = slice(c * HPC * (DN + DV), (c + 1) * HPC * (DN + DV))
        hrows = slice(c * HPC * DV, (c + 1) * HPC * DV)
        in_maps.append({
            "hsT": np.ascontiguousarray(hsT[:, r]),
            "cosr": np.ascontiguousarray(cos2[r]),
            "sinr": np.ascontiguousarray(sin2[r]),
            "wqa": wq_a.astype(bf),
            "wkva": wkva,
            "wqb": wqb,
            "wkvb": np.ascontiguousarray(wkvb[:, hcols]),
            "wo": np.ascontiguousarray(wob[hrows]),
        })
    return in_maps


def kernel(**inputs) -> np.ndarray:
    if "nc" not in _CACHED:
        _CACHED["nc"] = build()
    nc = _CACHED["nc"]
    in_maps = _prep(**inputs)
    res = run_bass_kernel_spmd(nc, in_maps, list(range(NC)))
    out = np.zeros((S, H), np.float32)
    for c in range(NC):
        out += res.results[c]["out"].astype(np.float32)
    return out.reshape(B, S, H)


if __name__ == "__main__":
    rng = np.random.RandomState(0)
    ins = {
        "hidden_states": rng.randn(B, S, H).astype(np.float32),
        "cos": rng.rand(B, S, DR).astype(np.float32),
        "sin": rng.rand(B, S, DR).astype(np.float32),
        "wq_a": (rng.randn(H, QLR) * 0.02).astype(np.float32),
        "q_ln": np.ones(QLR, np.float32),
        "wq_b": (rng.randn(QLR, NH * DQK) * 0.02).astype(np.float32),
        "wkv_a": (rng.randn(H, KVLR + DR) * 0.02).astype(np.float32),
        "kv_ln": np.ones(KVLR, np.float32),
        "wkv_b": (rng.randn(KVLR, NH * (DN + DV)) * 0.02).astype(np.float32),
        "wo": (rng.randn(NH * DV, H) * 0.02).astype(np.float32),
    }
    out = kernel(**ins)
    print("kernel out", out.shape, out.dtype, np.abs(out).mean())


# revision 20
# speedup vs baseline: 1.3417x; 1.0132x over previous
"""DeepseekV32 MLA-style attention on 8 Trainium2 NeuronCores (Bass/Tile).

Sharding: row shard (256 rows/core) for the low-rank A projections and q_b;
head shard (2 heads/core) for kv_b expansion + attention + o_proj.  Host
prep: hidden is pre-transposed (hsT input), rope columns of wkv_a / wq_b are
pre-permuted so the de-interleave is free, wo is sliced per-core by head,
layernorm weights and softmax scale are folded into the B projections.

Exactly two collectives (they serialize on the collective engine): AllGather
of X^T=[ckv_normed; roped k_rot] and AllToAll of q (row-shard ->
head-shard).  The output projection is computed per-head (a partial over all
rows) and the 8 partials are summed on the host, which removes the output
collective entirely.

Attention computes scores TRANSPOSED (k on partitions, q on free dim):
probsT = exp(scoresT) feeds the PV matmul directly (no transposes, no
PSUM->SBUF probs copies).  The softmax denominator comes from a ones-vector
matmul accumulated alongside PV; normalization is applied while draining
attT via partition_broadcast of the reciprocal.  o_proj is interleaved into
the attention stream as each 512-column block of both heads completes.

All matmuls run in bf16 with fp32 PSUM accumulation; softmax and rmsnorm
statistics are fp32.
"""
import sys

sys.path.insert(0, "/opt/trn_rl_repo")

import numpy as np
import ml_dtypes
from contextlib import ExitStack

import concourse.bass as bass
import concourse.tile as tile
import concourse.mybir as mybir
from concourse import bacc
from concourse.masks import make_identity
from concourse.bass_utils import run_bass_kernel_spmd

BF16 = mybir.dt.bfloat16
F32 = mybir.dt.float32
AF = mybir.ActivationFunctionType

NC = 8            # cores
B, S, H = 1, 2048, 2048
NH = 16           # heads
QLR = 1536        # q lora rank
KVLR = 512        # kv lora rank
DR = 64           # rope dim
DN = 128          # nope dim
DV = 128          # v dim
DQK = DN + DR     # 192
EPS = 1e-6
HPC = NH // NC    # heads per core = 2
SPC = S // NC     # seq rows per core = 256
ST = SPC // 128   # row tiles per core = 2
NEG = -1e30       # causal mask fill
NKT = KVLR // 128  # 4
HW = DR // 2      # 32

_CACHED = {}


def _ts(i, n):
    return slice(i * n, (i + 1) * n)


def _chunked(dram_ap, nchunk, rows, cols):
    """AP reading `nchunk` consecutive [rows, cols] row-blocks of a 2-D dram
    tensor as one [rows, nchunk, cols] transfer."""
    return bass.AP(tensor=dram_ap.tensor, offset=0,
                   ap=[[cols, rows], [rows * cols, nchunk], [1, cols]])


def build():
    nc = bacc.Bacc("TRN2", target_bir_lowering=False, debug=False,
                   num_devices=NC)

    # ---- kernel I/O (per-core shards / replicated weights) ----
    hsT_d = nc.dram_tensor("hsT", [H, SPC], BF16, kind="ExternalInput").ap()
    cos_d = nc.dram_tensor("cosr", [SPC, DR], F32, kind="ExternalInput").ap()
    sin_d = nc.dram_tensor("sinr", [SPC, DR], F32, kind="ExternalInput").ap()
    wqa_d = nc.dram_tensor("wqa", [H, QLR], BF16, kind="ExternalInput").ap()
    wkva_d = nc.dram_tensor("wkva", [H, KVLR + DR], BF16,
                            kind="ExternalInput").ap()
    wqb_d = nc.dram_tensor("wqb", [QLR, NH * DQK], BF16,
                           kind="ExternalInput").ap()
    wkvb_d = nc.dram_tensor("wkvb", [KVLR, HPC * (DN + DV)], BF16,
                            kind="ExternalInput").ap()
    wo_d = nc.dram_tensor("wo", [HPC * DV, H], BF16, kind="ExternalInput").ap()
    # per-core output: partial o_proj (this core's 2 heads) over ALL rows
    out_d = nc.dram_tensor("out", [S, H], BF16, kind="ExternalOutput").ap()

    # ---- collective buffers ----
    ag_in = nc.dram_tensor("ag_in", [KVLR + DR, SPC], BF16).ap()
    ag_out = nc.dram_tensor("ag_out", [NC * (KVLR + DR), SPC], BF16,
                            addr_space="Shared").ap()
    a2aq_in = nc.dram_tensor("a2aq_in", [S, HPC * DQK], BF16).ap()
    a2aq_out = nc.dram_tensor("a2aq_out", [S, HPC * DQK], BF16).ap()

    rg = [list(range(NC))]
    nh = H // 128

    with tile.TileContext(nc) as tc, ExitStack() as ctx:
        singles = ctx.enter_context(tc.tile_pool(name="singles", bufs=1))
        small = ctx.enter_context(tc.tile_pool(name="small", bufs=4))

        ident = singles.tile([128, 128], BF16)
        make_identity(nc, ident)
        eps_t = singles.tile([128, 1], F32)
        nc.vector.memset(eps_t, float(EPS))
        ones_bf = singles.tile([128, 1], BF16)
        nc.vector.memset(ones_bf, 1.0)
        # cmaskT[k, q] = 0 if k <= q else NEG (diagonal block of scoresT)
        cmaskT = singles.tile([128, 128], F32)
        nc.gpsimd.memset(cmaskT, 0.0)
        nc.gpsimd.affine_select(
            out=cmaskT, in_=cmaskT, compare_op=mybir.AluOpType.is_ge,
            fill=NEG, base=0, pattern=[[1, 128]], channel_multiplier=-1)

        hsT_sb = []
        wkva_sb = []
        wop = ctx.enter_context(tc.tile_pool(name="wop", bufs=1))
        interph = ctx.enter_context(tc.tile_pool(name="interph", bufs=1))

        # =========== phase 1: ckv -> X^T -> AllGather ===========
        with tc.tile_pool(name="ph1", bufs=1) as ph1, \
             tc.tile_pool(name="ps1", bufs=2, space="PSUM") as ps1:
            # batched input DMAs (one transfer per half-tensor; the SP
            # queue's per-DMA dispatch cost would otherwise pace the phase)
            wkva_all = ph1.tile([128, nh, KVLR + DR], BF16, tag="wkva",
                                name="wkva_all")
            hsT_all = interph.tile([128, nh, SPC], BF16, tag="hsT",
                                   name="hsT_all")
            for ch in range(2):
                hh = nh // 2
                src = bass.AP(tensor=hsT_d.tensor, offset=ch * hh * 128 * SPC,
                              ap=[[SPC, 128], [128 * SPC, hh], [1, SPC]])
                nc.sync.dma_start(out=hsT_all[:, _ts(ch, hh), :], in_=src)
                src = bass.AP(tensor=wkva_d.tensor,
                              offset=ch * hh * 128 * (KVLR + DR),
                              ap=[[KVLR + DR, 128], [128 * (KVLR + DR), hh],
                                  [1, KVLR + DR]])
                nc.sync.dma_start(out=wkva_all[:, _ts(ch, hh), :], in_=src)
            wkva_sb = [wkva_all[:, ht, :] for ht in range(nh)]
            hsT_sb = [hsT_all[:, ht, :] for ht in range(nh)]

            cos_all = singles.tile([128, ST, DR], F32, name="cos_all")
            sin_all = singles.tile([128, ST, DR], F32, name="sin_all")
            nc.sync.dma_start(out=cos_all, in_=_chunked(cos_d, ST, 128, DR))
            nc.sync.dma_start(out=sin_all, in_=_chunked(sin_d, ST, 128, DR))
            cos_sb = [cos_all[:, st, :] for st in range(ST)]
            sin_sb = [sin_all[:, st, :] for st in range(ST)]

            ckv_ps = [ps1.tile([128, KVLR], F32, tag=f"ckv{st}", bufs=1,
                               name="ckv_p") for st in range(ST)]
            rope_ps = [ps1.tile([128, DR], F32, tag=f"ckr{st}", bufs=1,
                                name="rope_p") for st in range(ST)]
            for ht in range(nh):
                for st in range(ST):
                    nc.tensor.matmul(out=ckv_ps[st],
                                     lhsT=hsT_sb[ht][:, _ts(st, 128)],
                                     rhs=wkva_sb[ht][:, 0:KVLR],
                                     start=(ht == 0), stop=(ht == nh - 1))
            for ht in range(nh):
                for st in range(ST):
                    nc.tensor.matmul(out=rope_ps[st],
                                     lhsT=hsT_sb[ht][:, _ts(st, 128)],
                                     rhs=wkva_sb[ht][:, KVLR:KVLR + DR],
                                     start=(ht == 0), stop=(ht == nh - 1))

            for st in range(ST):
                # rmsnorm over KVLR, stats straight off PSUM
                sq = small.tile([128, KVLR], F32, tag="sqscr", bufs=2,
                                name="sq")
                ssq = small.tile([128, 1], F32, tag="ssq", name="ssq")
                nc.scalar.activation(out=sq, in_=ckv_ps[st], func=AF.Square,
                                     accum_out=ssq)
                rstd = small.tile([128, 1], F32, tag="rstd", name="rstd")
                nc.scalar.activation(out=rstd, in_=ssq, func=AF.Sqrt,
                                     scale=1.0 / KVLR, bias=eps_t)
                nc.vector.reciprocal(out=rstd, in_=rstd)
                cn_t = ph1.tile([128, KVLR], BF16, tag=f"ckvn{st}",
                                name="cn_t")
                nc.vector.tensor_scalar_mul(cn_t, ckv_ps[st], rstd)
                # rope on k_rot (weights pre-permuted -> contiguous halves)
                kr_t = ph1.tile([128, DR], BF16, tag=f"krot{st}", name="kr_t")
                t0 = small.tile([128, HW], F32, tag="krs0", bufs=2, name="t0")
                t1 = small.tile([128, HW], F32, tag="krs1", bufs=2, name="t1")
                xe, xo = rope_ps[st][:, 0:HW], rope_ps[st][:, HW:DR]
                nc.vector.tensor_mul(t0, xe, cos_sb[st][:, 0:HW])
                nc.vector.tensor_mul(t1, xo, sin_sb[st][:, 0:HW])
                nc.vector.tensor_sub(kr_t[:, 0:HW], t0, t1)
                nc.vector.tensor_mul(t0, xo, cos_sb[st][:, HW:DR])
                nc.vector.tensor_mul(t1, xe, sin_sb[st][:, HW:DR])
                nc.vector.tensor_add(kr_t[:, HW:DR], t0, t1)

                # transpose [128, 512+64] -> X^T, stage with 2 DMAs per st
                xp = ps1.tile([128, KVLR], BF16, tag="xp", name="xp")
                for kt in range(NKT):
                    nc.tensor.transpose(out=xp[:, _ts(kt, 128)],
                                        in_=cn_t[:, _ts(kt, 128)],
                                        identity=ident)
                x_t = small.tile([128, NKT, 128], BF16, tag="xT", bufs=2,
                                 name="x_t")
                if st == 0:
                    nc.vector.tensor_copy(out=x_t,
                                          in_=xp.rearrange(
                                              "p (k c) -> p k c", k=NKT))
                else:
                    nc.scalar.copy(out=x_t,
                                   in_=xp.rearrange("p (k c) -> p k c",
                                                    k=NKT))
                dst = bass.AP(tensor=ag_in.tensor, offset=st * 128,
                              ap=[[SPC, 128], [128 * SPC, NKT], [1, 128]])
                nc.gpsimd.dma_start(out=dst, in_=x_t)
                pr_t = ps1.tile([64, 128], BF16, tag="tpr", name="pr_t")
                nc.tensor.transpose(out=pr_t, in_=kr_t, identity=ident)
                xr_t = small.tile([64, 128], BF16, tag="xTr", name="xr_t")
                nc.vector.tensor_copy(out=xr_t, in_=pr_t)
                nc.gpsimd.dma_start(out=ag_in[KVLR:KVLR + DR, _ts(st, 128)],
                                    in_=xr_t)

            # ---- collective 1: AllGather X^T ----
            nc.gpsimd.collective_compute(
                "AllGather", mybir.AluOpType.bypass, replica_groups=rg,
                ins=[ag_in.opt()], outs=[ag_out.opt()])

        # =========== phase 2: cq -> q_b -> rope -> AllToAll q ===========
        wqb_stack = ExitStack()
        wqbp = wqb_stack.enter_context(tc.tile_pool(name="wqbp", bufs=1))
        wkvb_sb = []
        wo_sb = []
        with tc.tile_pool(name="ph2", bufs=1) as ph2:
            # wqa in 4 chunks so cq overlaps the transfer
            wqa_sb = []
            for cg in range(4):
                wq_t = ph2.tile([128, 4, QLR], BF16, tag=f"wqa{cg}",
                                name="wq_t")
                src = bass.AP(tensor=wqa_d.tensor, offset=cg * 4 * 128 * QLR,
                              ap=[[QLR, 128], [128 * QLR, 4], [1, QLR]])
                nc.sync.dma_start(out=wq_t, in_=src)
                wqa_sb += [wq_t[:, i, :] for i in range(4)]
            wkvb_all = interph.tile([128, NKT, HPC * (DN + DV)], BF16,
                                    tag="wkvb", name="wkvb_all")
            nc.sync.dma_start(out=wkvb_all,
                              in_=_chunked(wkvb_d, NKT, 128,
                                           HPC * (DN + DV)))
            wkvb_sb = [wkvb_all[:, kt, :] for kt in range(NKT)]
            wqb_sb = []
            for cg in range(6):
                wb_t = wqbp.tile([128, 2, NH * DQK], BF16, tag=f"wqb{cg}",
                                 name="wb_t")
                src = bass.AP(tensor=wqb_d.tensor,
                              offset=cg * 2 * 128 * NH * DQK,
                              ap=[[NH * DQK, 128], [128 * NH * DQK, 2],
                                  [1, NH * DQK]])
                nc.sync.dma_start(out=wb_t, in_=src)
                wqb_sb += [wb_t[:, i, :] for i in range(2)]
            wo_all = wop.tile([128, HPC, H], BF16, tag="wo", name="wo_all")
            nc.sync.dma_start(out=wo_all, in_=_chunked(wo_d, HPC, 128, H))
            wo_sb = [wo_all[:, h, :] for h in range(HPC)]

            # cq: ht outer (stream wqa), both row-tiles in parallel
            s2a = ExitStack()
            ps2a = s2a.enter_context(tc.tile_pool(name="ps2a", bufs=1,
                                                  space="PSUM"))
            cq_ps = [[ps2a.tile([128, 512], F32, tag=f"cq{st}_{rb}", bufs=1,
                                name="cq_p") for rb in range(QLR // 512)]
                     for st in range(ST)]
            for ht in range(nh):
                for st in range(ST):
                    for rb in range(QLR // 512):
                        nc.tensor.matmul(out=cq_ps[st][rb],
                                         lhsT=hsT_sb[ht][:, _ts(st, 128)],
                                         rhs=wqa_sb[ht][:, _ts(rb, 512)],
                                         start=(ht == 0), stop=(ht == nh - 1))
            cqn_bf = []
            for st in range(ST):
                ssqs = []
                for rb in range(QLR // 512):
                    sqq = small.tile([128, 512], F32, tag="sqq", bufs=2,
                                     name="sqq")
                    ssq = small.tile([128, 1], F32, tag="ssq3", bufs=6,
                                     name="ssq")
                    nc.scalar.activation(out=sqq, in_=cq_ps[st][rb],
                                         func=AF.Square, accum_out=ssq)
                    ssqs.append(ssq)
                nc.vector.tensor_add(ssqs[0], ssqs[0], ssqs[1])
                nc.vector.tensor_add(ssqs[0], ssqs[0], ssqs[2])
                rstd = small.tile([128, 1], F32, tag="rstd", name="rstd")
                nc.scalar.activation(out=rstd, in_=ssqs[0], func=AF.Sqrt,
                                     scale=1.0 / QLR, bias=eps_t)
                nc.vector.reciprocal(out=rstd, in_=rstd)
                cn_t = ph2.tile([128, QLR], BF16, tag=f"cqn{st}", name="cn_t")
                for rb in range(QLR // 512):
                    nc.vector.tensor_scalar_mul(cn_t[:, _ts(rb, 512)],
                                                cq_ps[st][rb], rstd)
                cqn_bf.append(cn_t)

            # transpose cqn -> cqnT [1536, 256] (batched drains, alternating
            # engines)
            cqnT = []
            for rt in range(QLR // 128):
                cT_t = ph2.tile([128, SPC], BF16, tag=f"cqnT{rt}",
                                name="cT_t")
                p_t = ps2a.tile([128, SPC], BF16, tag="tp", bufs=2,
                                name="p_t")
                for st in range(ST):
                    nc.tensor.transpose(out=p_t[:, _ts(st, 128)],
                                        in_=cqn_bf[st][:, _ts(rt, 128)],
                                        identity=ident)
                if rt % 2 == 0:
                    nc.scalar.copy(out=cT_t, in_=p_t)
                else:
                    nc.vector.tensor_copy(out=cT_t, in_=p_t)
                cqnT.append(cT_t)
            s2a.close()

            # q_b per row-tile; psum in head-pair blocks of 384 cols so the
            # rope slicing never crosses a PSUM tile boundary
            s2b = ExitStack()
            ps2b = s2b.enter_context(tc.tile_pool(name="ps2b", bufs=1,
                                                  space="PSUM"))
            nr = QLR // 128
            for st in range(ST):
                q_ps = [ps2b.tile([128, HPC * DQK], F32, tag=f"qb{nb}",
                                  bufs=1, name="q_p") for nb in range(NC)]
                for rt in range(nr):
                    for nb in range(NC):
                        nc.tensor.matmul(out=q_ps[nb],
                                         lhsT=cqnT[rt][:, _ts(st, 128)],
                                         rhs=wqb_sb[rt][:, _ts(nb, HPC * DQK)],
                                         start=(rt == 0), stop=(rt == nr - 1))
                # rope + bf16 pack: nope copies on Act, rope muls on DVE
                q_bf = ph2.tile([128, NH, DQK], BF16, tag=f"qbf{st}",
                                name="q_bf")
                for nb in range(NC):
                    qv = q_ps[nb].rearrange("p (h d) -> p h d", h=HPC)
                    dst = q_bf[:, nb * HPC:(nb + 1) * HPC, :]
                    nc.scalar.copy(out=dst[:, :, 0:DN], in_=qv[:, :, 0:DN])

                    def _bc(t, lo, hi):
                        return bass.AP(
                            tensor=t.tensor, offset=t.offset + lo,
                            ap=[list(t.ap[0]), [0, HPC], [1, hi - lo]])
                    cs, sn = cos_sb[st], sin_sb[st]
                    xe, xo = qv[:, :, DN:DN + HW], qv[:, :, DN + HW:DQK]
                    t0 = small.tile([128, HPC, HW], F32, tag="qrs0", bufs=2,
                                    name="t0")
                    t1 = small.tile([128, HPC, HW], F32, tag="qrs1", bufs=2,
                                    name="t1")
                    nc.vector.tensor_mul(t0, xe, _bc(cs, 0, HW))
                    nc.vector.tensor_mul(t1, xo, _bc(sn, 0, HW))
                    nc.vector.tensor_sub(dst[:, :, DN:DN + HW], t0, t1)
                    nc.vector.tensor_mul(t0, xo, _bc(cs, HW, DR))
                    nc.vector.tensor_mul(t1, xe, _bc(sn, HW, DR))
                    nc.vector.tensor_add(dst[:, :, DN + HW:DQK], t0, t1)
                # stage the whole row-tile with one DMA (8 dest chunks)
                dst = bass.AP(tensor=a2aq_in.tensor,
                              offset=st * 128 * HPC * DQK,
                              ap=[[HPC * DQK, 128], [SPC * HPC * DQK, NC],
                                  [1, HPC * DQK]])
                nc.gpsimd.dma_start(out=dst, in_=q_bf)
            s2b.close()
            # ---- collective 2: AllToAll q ----
            nc.gpsimd.collective_compute(
                "AllToAll", mybir.AluOpType.bypass, replica_groups=rg,
                ins=[a2aq_in.opt()], outs=[a2aq_out.opt()])
        wqb_stack.close()

        # =========== phase 3: k/v expansion + qT ===========
        with tc.tile_pool(name="ph3", bufs=1) as ph3, \
             tc.tile_pool(name="ph3b", bufs=4) as ph3b:
            s3 = ExitStack()
            ps3 = s3.enter_context(tc.tile_pool(name="ps3", bufs=2,
                                                space="PSUM"))
            krT = ph3.tile([64, NC, SPC], BF16, tag="krT", name="krT")
            src = bass.AP(tensor=ag_out.tensor, offset=KVLR * SPC,
                          ap=[[SPC, 64], [(KVLR + DR) * SPC, NC], [1, SPC]])
            nc.sync.dma_start(out=krT, in_=src)
            krTf = krT.rearrange("p g c -> p (g c)")

            kT = [ph3.tile([128, S], BF16, tag=f"kT{h}", name="kT_t")
                  for h in range(HPC)]
            v_sb = [[ph3.tile([128, DV], BF16, tag=f"v{h}_{kc}", name="v_t")
                     for kc in range(S // 128)] for h in range(HPC)]
            for g in range(NC):
                xk_t = ph3b.tile([128, NKT, SPC], BF16, tag="xk", bufs=3,
                                 name="xk_t")
                src = bass.AP(tensor=ag_out.tensor,
                              offset=g * (KVLR + DR) * SPC,
                              ap=[[SPC, 128], [128 * SPC, NKT], [1, SPC]])
                nc.sync.dma_start(out=xk_t, in_=src)
                xk = [xk_t[:, kt, :] for kt in range(NKT)]
                kps = [ps3.tile([128, SPC], F32, tag="mmk", bufs=2, name="kp")
                       for _ in range(HPC)]
                for kt in range(NKT):
                    for h in range(HPC):
                        nc.tensor.matmul(
                            out=kps[h],
                            lhsT=wkvb_sb[kt][:, h * (DN + DV):
                                             h * (DN + DV) + DN],
                            rhs=xk[kt], start=(kt == 0), stop=(kt == NKT - 1))
                for h in range(HPC):
                    if h == 0:
                        nc.scalar.copy(out=kT[h][:, _ts(g, SPC)], in_=kps[h])
                    else:
                        nc.vector.tensor_copy(out=kT[h][:, _ts(g, SPC)],
                                              in_=kps[h])
                for sub in range(ST):
                    vps = [ps3.tile([128, DV], F32, tag="mmv", bufs=2,
                                    name="vp") for _ in range(HPC)]
                    for kt in range(NKT):
                        for h in range(HPC):
                            nc.tensor.matmul(
                                out=vps[h], lhsT=xk[kt][:, _ts(sub, 128)],
                                rhs=wkvb_sb[kt][:, h * (DN + DV) + DN:
                                                (h + 1) * (DN + DV)],
                                start=(kt == 0), stop=(kt == NKT - 1))
                    for h in range(HPC):
                        nc.vector.tensor_copy(out=v_sb[h][g * ST + sub],
                                              in_=vps[h])

            # q^T per head from the AllToAll (batched loads + drains)
            qTn = [ph3.tile([128, S], BF16, tag=f"qTn{h}", name="qTn_t")
                   for h in range(HPC)]
            qTr = [ph3.tile([64, S], BF16, tag=f"qTr{h}", name="qTr_t")
                   for h in range(HPC)]
            for q4 in range(4):
                qblk = ph3b.tile([128, 4, HPC * DQK], BF16, tag="qblk",
                                 bufs=2, name="qblk")
                src = bass.AP(tensor=a2aq_out.tensor,
                              offset=q4 * 4 * 128 * HPC * DQK,
                              ap=[[HPC * DQK, 128], [128 * HPC * DQK, 4],
                                  [1, HPC * DQK]])
                nc.sync.dma_start(out=qblk, in_=src)
                for h in range(HPC):
                    pn = ps3.tile([128, 512], BF16, tag="tqn", bufs=2,
                                  name="pn")
                    pr = ps3.tile([64, 512], BF16, tag="tqr", bufs=2,
                                  name="pr")
                    for i in range(4):
                        nc.tensor.transpose(
                            out=pn[:, _ts(i, 128)],
                            in_=qblk[:, i, h * DQK:h * DQK + DN],
                            identity=ident)
                        nc.tensor.transpose(
                            out=pr[:, _ts(i, 128)],
                            in_=qblk[:, i, h * DQK + DN:(h + 1) * DQK],
                            identity=ident)
                    if h == 0:
                        nc.scalar.copy(out=qTn[h][:, _ts(q4, 512)], in_=pn)
                        nc.vector.tensor_copy(out=qTr[h][:, _ts(q4, 512)],
                                              in_=pr)
                    else:
                        nc.vector.tensor_copy(out=qTn[h][:, _ts(q4, 512)],
                                              in_=pn)
                        nc.scalar.copy(out=qTr[h][:, _ts(q4, 512)], in_=pr)
            s3.close()

            # ====== phase 4: attention (scoresT) + interleaved o_proj ======
            QB = 512
            NQB = S // QB
            attTn = [[None] * NQB for _ in range(HPC)]

            def oproj(qb, ps_pool, o_pool):
                """o_proj for q rows [qb*512, (qb+1)*512): both heads."""
                for sub in range(4):
                    qs = qb * 4 + sub
                    o_t = o_pool.tile([128, H], BF16, tag="osb", bufs=3,
                                      name="o_t")
                    for cb in range(H // 512):
                        op = ps_pool.tile([128, 512], F32, tag="op", bufs=2,
                                          name="op")
                        for h in range(HPC):
                            nc.tensor.matmul(
                                out=op,
                                lhsT=attTn[h][qb][:, _ts(sub, 128)],
                                rhs=wo_sb[h][:, _ts(cb, 512)],
                                start=(h == 0), stop=(h == HPC - 1))
                        if cb % 2 == 0:
                            nc.scalar.copy(out=o_t[:, _ts(cb, 512)], in_=op)
                        else:
                            nc.vector.tensor_copy(out=o_t[:, _ts(cb, 512)],
                                                  in_=op)
                    nc.sync.dma_start(out=out_d[_ts(qs, 128), :], in_=o_t)

            with tc.tile_pool(name="ps5", bufs=1, space="PSUM") as ps5, \
                 tc.tile_pool(name="ph5", bufs=1) as ph5:
                for qb in range(NQB):
                    for h in range(HPC):
                        # previous block's o_proj slots between the two head
                        # chains: its inputs are long-ready, so PE streams
                        # through it with no dependency stalls
                        if h == 1 and qb > 0:
                            oproj(qb - 1, ps5, ph5)
                        attp = ps5.tile([128, QB], F32, tag="attT", bufs=2,
                                        name="attp")
                        denp = ps5.tile([1, QB], F32, tag="den", bufs=2,
                                        name="denp")
                        nkc = 4 * qb + 4
                        # software-pipelined: PV/den of kc trail the score
                        # matmuls of kc+1 so PE never waits on exp
                        probs = [None] * nkc

                        def scores(kc):
                            off = max(0, (kc - 4 * qb) * 128)
                            scp = ps5.tile([128, QB], F32, tag="scT", bufs=2,
                                           name="scp")
                            nc.tensor.matmul(
                                out=scp[:, off:QB],
                                lhsT=kT[h][:, _ts(kc, 128)],
                                rhs=qTn[h][:, qb * QB + off:(qb + 1) * QB],
                                start=True, stop=False)
                            nc.tensor.matmul(
                                out=scp[:, off:QB],
                                lhsT=krTf[:, _ts(kc, 128)],
                                rhs=qTr[h][:, qb * QB + off:(qb + 1) * QB],
                                start=False, stop=True)
                            if kc >= 4 * qb:
                                nc.vector.tensor_add(scp[:, off:off + 128],
                                                     scp[:, off:off + 128],
                                                     cmaskT)
                            pt = ph3b.tile([128, QB], BF16, tag="probsT",
                                           bufs=4, name="probsT")
                            if off > 0:
                                nc.vector.memset(pt[:, 0:off], 0.0)
                            nc.scalar.activation(out=pt[:, off:QB],
                                                 in_=scp[:, off:QB],
                                                 func=AF.Exp)
                            probs[kc] = pt

                        def pv(kc):
                            nc.tensor.matmul(out=attp, lhsT=v_sb[h][kc],
                                             rhs=probs[kc],
                                             start=(kc == 0),
                                             stop=(kc == nkc - 1))
                            nc.tensor.matmul(out=denp, lhsT=ones_bf,
                                             rhs=probs[kc],
                                             start=(kc == 0),
                                             stop=(kc == nkc - 1))

                        scores(0)
                        for kc in range(1, nkc):
                            scores(kc)
                            pv(kc - 1)
                        pv(nkc - 1)

                        # normalize while draining attT
                        rec = small.tile([1, QB], F32, tag="rec", bufs=4,
                                         name="rec")
                        nc.vector.reciprocal(out=rec, in_=denp)
                        bca = small.tile([128, QB], F32, tag="bca", bufs=2,
                                         name="bca")
                        nc.gpsimd.partition_broadcast(bca, rec)
                        a_t = ph5.tile([128, QB], BF16, tag=f"attn{h}_{qb}",
                                       name="a_t")
                        nc.vector.tensor_mul(a_t, attp, bca)
                        attTn[h][qb] = a_t
                oproj(NQB - 1, ps5, ph5)

    nc.compile()
    return nc


def _prep(hidden_states, cos, sin, wq_a, q_ln, wq_b, wkv_a, kv_ln, wkv_b, wo):
    """Host-side sharding + weight prep: pre-transpose hidden, fold layernorm
    weights + softmax scale into the B projections, pre-permute rope columns
    (de-interleave), slice wo by head, cast to bf16."""
    bf = ml_dtypes.bfloat16
    hsT = np.ascontiguousarray(hidden_states.reshape(S, H).T.astype(bf))
    cos2 = np.ascontiguousarray(cos.reshape(S, DR).astype(np.float32))
    sin2 = np.ascontiguousarray(sin.reshape(S, DR).astype(np.float32))

    # de-interleave permutation for a 64-wide rope slice
    perm = np.concatenate([np.arange(0, DR, 2), np.arange(1, DR, 2)])

    wkva = np.array(wkv_a, copy=True)
    wkva[:, KVLR:] = wkva[:, KVLR:][:, perm]
    wkva = wkva.astype(bf)

    scale = np.float32(DQK) ** np.float32(-0.5)
    wqb = np.asarray(wq_b * q_ln[:, None] * scale)
    wqb = wqb.reshape(QLR, NH, DQK)
    wqb = np.concatenate([wqb[:, :, :DN], wqb[:, :, DN:][:, :, perm]],
                         axis=2).reshape(QLR, NH * DQK).astype(bf)

    wkvb = (wkv_b * kv_ln[:, None]).astype(bf)
    wob = wo.astype(bf)

    in_maps = []
    for c in range(NC):
        r = slice(c * SPC, (c + 1) * SPC)
        hcols = slice(c * HPC * (DN + DV), (c + 1) * HPC * (DN + DV))
        hrows = slice(c * HPC * DV, (c + 1) * HPC * DV)
        in_maps.append({
            "hsT": np.ascontiguousarray(hsT[:, r]),
            "cosr": np.ascontiguousarray(cos2[r]),
            "sinr": np.ascontiguousarray(sin2[r]),
            "wqa": wq_a.astype(bf),
            "wkva": wkva,
            "wqb": wqb,
            "wkvb": np.ascontiguousarray(wkvb[:, hcols]),
            "wo": np.ascontiguousarray(wob[hrows]),
        })
    return in_maps


def kernel(**inputs) -> np.ndarray:
    if "nc" not in _CACHED:
        _CACHED["nc"] = build()
    nc = _CACHED["nc"]
    in_maps = _prep(**inputs)
    res = run_bass_kernel_spmd(nc, in_maps, list(range(NC)))
    out = np.zeros((S, H), np.float32)
    for c in range(NC):
        out += res.results[c]["out"].astype(np.float32)
    return out.reshape(B, S, H)


if __name__ == "__main__":
    rng = np.random.RandomState(0)
    ins = {
        "hidden_states": rng.randn(B, S, H).astype(np.float32),
        "cos": rng.rand(B, S, DR).astype(np.float32),
        "sin": rng.rand(B, S, DR).astype(np.float32),
        "wq_a": (rng.randn(H, QLR) * 0.02).astype(np.float32),
        "q_ln": np.ones(QLR, np.float32),
        "wq_b": (rng.randn(QLR, NH * DQK) * 0.02).astype(np.float32),
        "wkv_a": (rng.randn(H, KVLR + DR) * 0.02).astype(np.float32),
        "kv_ln": np.ones(KVLR, np.float32),
        "wkv_b": (rng.randn(KVLR, NH * (DN + DV)) * 0.02).astype(np.float32),
        "wo": (rng.randn(NH * DV, H) * 0.02).astype(np.float32),
    }
    out = kernel(**ins)
    print("kernel out", out.shape, out.dtype, np.abs(out).mean())


# revision 21
# speedup vs baseline: 1.3754x; 1.0251x over previous
"""DeepseekV32 MLA-style attention on 8 Trainium2 NeuronCores (Bass/Tile).

Sharding: row shard (256 rows/core) for the low-rank A projections and q_b;
head shard (2 heads/core) for kv_b expansion + attention + o_proj.  Host
prep: hidden is pre-transposed (hsT input), rope columns of wkv_a / wq_b are
pre-permuted so the de-interleave is free, wo is sliced per-core by head,
layernorm weights and softmax scale are folded into the B projections.

Exactly two collectives (they serialize on the collective engine): AllGather
of X^T=[ckv_normed; roped k_rot] and AllToAll of q (row-shard ->
head-shard).  The output projection is computed per-head (a partial over all
rows) and the 8 partials are summed on the host, which removes the output
collective entirely.

Attention computes scores TRANSPOSED (k on partitions, q on free dim):
probsT = exp(scoresT) feeds the PV matmul directly (no transposes, no
PSUM->SBUF probs copies).  The softmax denominator comes from a ones-vector
matmul accumulated alongside PV; normalization is applied while draining
attT via partition_broadcast of the reciprocal.  o_proj is interleaved into
the attention stream as each 512-column block of both heads completes.

All matmuls run in bf16 with fp32 PSUM accumulation; softmax and rmsnorm
statistics are fp32.
"""
import sys

sys.path.insert(0, "/opt/trn_rl_repo")

import numpy as np
import ml_dtypes
from contextlib import ExitStack

import concourse.bass as bass
import concourse.tile as tile
import concourse.mybir as mybir
from concourse import bacc
from concourse.masks import make_identity
from concourse.bass_utils import run_bass_kernel_spmd

BF16 = mybir.dt.bfloat16
F32 = mybir.dt.float32
AF = mybir.ActivationFunctionType

NC = 8            # cores
B, S, H = 1, 2048, 2048
NH = 16           # heads
QLR = 1536        # q lora rank
KVLR = 512        # kv lora rank
DR = 64           # rope dim
DN = 128          # nope dim
DV = 128          # v dim
DQK = DN + DR     # 192
EPS = 1e-6
HPC = NH // NC    # heads per core = 2
SPC = S // NC     # seq rows per core = 256
ST = SPC // 128   # row tiles per core = 2
NEG = -1e30       # causal mask fill
NKT = KVLR // 128  # 4
HW = DR // 2      # 32

_CACHED = {}


def _ts(i, n):
    return slice(i * n, (i + 1) * n)


def _chunked(dram_ap, nchunk, rows, cols):
    """AP reading `nchunk` consecutive [rows, cols] row-blocks of a 2-D dram
    tensor as one [rows, nchunk, cols] transfer."""
    return bass.AP(tensor=dram_ap.tensor, offset=0,
                   ap=[[cols, rows], [rows * cols, nchunk], [1, cols]])


def build():
    nc = bacc.Bacc("TRN2", target_bir_lowering=False, debug=False,
                   num_devices=NC)

    # ---- kernel I/O (per-core shards / replicated weights) ----
    hsT_d = nc.dram_tensor("hsT", [H, SPC], BF16, kind="ExternalInput").ap()
    cos_d = nc.dram_tensor("cosr", [SPC, DR], F32, kind="ExternalInput").ap()
    sin_d = nc.dram_tensor("sinr", [SPC, DR], F32, kind="ExternalInput").ap()
    wqa_d = nc.dram_tensor("wqa", [H, QLR], BF16, kind="ExternalInput").ap()
    wkva_d = nc.dram_tensor("wkva", [H, KVLR + DR], BF16,
                            kind="ExternalInput").ap()
    wqb_d = nc.dram_tensor("wqb", [QLR, NH * DQK], BF16,
                           kind="ExternalInput").ap()
    wkvb_d = nc.dram_tensor("wkvb", [KVLR, HPC * (DN + DV)], BF16,
                            kind="ExternalInput").ap()
    wo_d = nc.dram_tensor("wo", [HPC * DV, H], BF16, kind="ExternalInput").ap()
    # per-core output: partial o_proj (this core's 2 heads) over ALL rows
    out_d = nc.dram_tensor("out", [S, H], BF16, kind="ExternalOutput").ap()

    # ---- collective buffers ----
    ag_in = nc.dram_tensor("ag_in", [KVLR + DR, SPC], BF16).ap()
    ag_out = nc.dram_tensor("ag_out", [NC * (KVLR + DR), SPC], BF16,
                            addr_space="Shared").ap()
    a2aq_in = nc.dram_tensor("a2aq_in", [S, HPC * DQK], BF16).ap()
    a2aq_out = nc.dram_tensor("a2aq_out", [S, HPC * DQK], BF16).ap()

    rg = [list(range(NC))]
    nh = H // 128

    with tile.TileContext(nc) as tc, ExitStack() as ctx:
        singles = ctx.enter_context(tc.tile_pool(name="singles", bufs=1))
        small = ctx.enter_context(tc.tile_pool(name="small", bufs=4))

        ident = singles.tile([128, 128], BF16)
        make_identity(nc, ident)
        eps_t = singles.tile([128, 1], F32)
        nc.vector.memset(eps_t, float(EPS))
        ones_bf = singles.tile([128, 1], BF16)
        nc.vector.memset(ones_bf, 1.0)
        # cmaskT[k, q] = 0 if k <= q else NEG (diagonal block of scoresT)
        cmaskT = singles.tile([128, 128], F32)
        nc.gpsimd.memset(cmaskT, 0.0)
        nc.gpsimd.affine_select(
            out=cmaskT, in_=cmaskT, compare_op=mybir.AluOpType.is_ge,
            fill=NEG, base=0, pattern=[[1, 128]], channel_multiplier=-1)

        hsT_sb = []
        wkva_sb = []
        wop = ctx.enter_context(tc.tile_pool(name="wop", bufs=1))
        interph = ctx.enter_context(tc.tile_pool(name="interph", bufs=1))

        # =========== phase 1: ckv -> X^T -> AllGather ===========
        with tc.tile_pool(name="ph1", bufs=1) as ph1, \
             tc.tile_pool(name="ps1", bufs=2, space="PSUM") as ps1:
            # batched input DMAs (one transfer per half-tensor; the SP
            # queue's per-DMA dispatch cost would otherwise pace the phase)
            wkva_all = ph1.tile([128, nh, KVLR + DR], BF16, tag="wkva",
                                name="wkva_all")
            hsT_all = interph.tile([128, nh, SPC], BF16, tag="hsT",
                                   name="hsT_all")
            for ch in range(2):
                hh = nh // 2
                src = bass.AP(tensor=hsT_d.tensor, offset=ch * hh * 128 * SPC,
                              ap=[[SPC, 128], [128 * SPC, hh], [1, SPC]])
                nc.sync.dma_start(out=hsT_all[:, _ts(ch, hh), :], in_=src)
                src = bass.AP(tensor=wkva_d.tensor,
                              offset=ch * hh * 128 * (KVLR + DR),
                              ap=[[KVLR + DR, 128], [128 * (KVLR + DR), hh],
                                  [1, KVLR + DR]])
                nc.sync.dma_start(out=wkva_all[:, _ts(ch, hh), :], in_=src)
            wkva_sb = [wkva_all[:, ht, :] for ht in range(nh)]
            hsT_sb = [hsT_all[:, ht, :] for ht in range(nh)]

            cos_all = singles.tile([128, ST, DR], F32, name="cos_all")
            sin_all = singles.tile([128, ST, DR], F32, name="sin_all")
            nc.sync.dma_start(out=cos_all, in_=_chunked(cos_d, ST, 128, DR))
            nc.sync.dma_start(out=sin_all, in_=_chunked(sin_d, ST, 128, DR))
            cos_sb = [cos_all[:, st, :] for st in range(ST)]
            sin_sb = [sin_all[:, st, :] for st in range(ST)]

            ckv_ps = [ps1.tile([128, KVLR], F32, tag=f"ckv{st}", bufs=1,
                               name="ckv_p") for st in range(ST)]
            rope_ps = [ps1.tile([128, DR], F32, tag=f"ckr{st}", bufs=1,
                                name="rope_p") for st in range(ST)]
            for ht in range(nh):
                for st in range(ST):
                    nc.tensor.matmul(out=ckv_ps[st],
                                     lhsT=hsT_sb[ht][:, _ts(st, 128)],
                                     rhs=wkva_sb[ht][:, 0:KVLR],
                                     start=(ht == 0), stop=(ht == nh - 1))
            for ht in range(nh):
                for st in range(ST):
                    nc.tensor.matmul(out=rope_ps[st],
                                     lhsT=hsT_sb[ht][:, _ts(st, 128)],
                                     rhs=wkva_sb[ht][:, KVLR:KVLR + DR],
                                     start=(ht == 0), stop=(ht == nh - 1))

            for st in range(ST):
                # rmsnorm over KVLR, stats straight off PSUM
                sq = small.tile([128, KVLR], F32, tag="sqscr", bufs=2,
                                name="sq")
                ssq = small.tile([128, 1], F32, tag="ssq", name="ssq")
                nc.scalar.activation(out=sq, in_=ckv_ps[st], func=AF.Square,
                                     accum_out=ssq)
                rstd = small.tile([128, 1], F32, tag="rstd", name="rstd")
                nc.scalar.activation(out=rstd, in_=ssq, func=AF.Sqrt,
                                     scale=1.0 / KVLR, bias=eps_t)
                nc.vector.reciprocal(out=rstd, in_=rstd)
                cn_t = ph1.tile([128, KVLR], BF16, tag=f"ckvn{st}",
                                name="cn_t")
                nc.vector.tensor_scalar_mul(cn_t, ckv_ps[st], rstd)
                # rope on k_rot (weights pre-permuted -> contiguous halves)
                kr_t = ph1.tile([128, DR], BF16, tag=f"krot{st}", name="kr_t")
                t0 = small.tile([128, HW], F32, tag="krs0", bufs=2, name="t0")
                t1 = small.tile([128, HW], F32, tag="krs1", bufs=2, name="t1")
                xe, xo = rope_ps[st][:, 0:HW], rope_ps[st][:, HW:DR]
                nc.vector.tensor_mul(t0, xe, cos_sb[st][:, 0:HW])
                nc.vector.tensor_mul(t1, xo, sin_sb[st][:, 0:HW])
                nc.vector.tensor_sub(kr_t[:, 0:HW], t0, t1)
                nc.vector.tensor_mul(t0, xo, cos_sb[st][:, HW:DR])
                nc.vector.tensor_mul(t1, xe, sin_sb[st][:, HW:DR])
                nc.vector.tensor_add(kr_t[:, HW:DR], t0, t1)

                # transpose [128, 512+64] -> X^T, stage with 2 DMAs per st
                xp = ps1.tile([128, KVLR], BF16, tag="xp", name="xp")
                for kt in range(NKT):
                    nc.tensor.transpose(out=xp[:, _ts(kt, 128)],
                                        in_=cn_t[:, _ts(kt, 128)],
                                        identity=ident)
                x_t = small.tile([128, NKT, 128], BF16, tag="xT", bufs=2,
                                 name="x_t")
                if st == 0:
                    nc.vector.tensor_copy(out=x_t,
                                          in_=xp.rearrange(
                                              "p (k c) -> p k c", k=NKT))
                else:
                    nc.scalar.copy(out=x_t,
                                   in_=xp.rearrange("p (k c) -> p k c",
                                                    k=NKT))
                dst = bass.AP(tensor=ag_in.tensor, offset=st * 128,
                              ap=[[SPC, 128], [128 * SPC, NKT], [1, 128]])
                nc.gpsimd.dma_start(out=dst, in_=x_t)
                pr_t = ps1.tile([64, 128], BF16, tag="tpr", name="pr_t")
                nc.tensor.transpose(out=pr_t, in_=kr_t, identity=ident)
                xr_t = small.tile([64, 128], BF16, tag="xTr", name="xr_t")
                nc.vector.tensor_copy(out=xr_t, in_=pr_t)
                nc.gpsimd.dma_start(out=ag_in[KVLR:KVLR + DR, _ts(st, 128)],
                                    in_=xr_t)

            # ---- collective 1: AllGather X^T ----
            nc.gpsimd.collective_compute(
                "AllGather", mybir.AluOpType.bypass, replica_groups=rg,
                ins=[ag_in.opt()], outs=[ag_out.opt()])

        # =========== phase 2: cq -> q_b -> rope -> AllToAll q ===========
        wqb_stack = ExitStack()
        wqbp = wqb_stack.enter_context(tc.tile_pool(name="wqbp", bufs=1))
        wkvb_sb = []
        wo_sb = []
        with tc.tile_pool(name="ph2", bufs=1) as ph2:
            # wqa in 4 chunks so cq overlaps the transfer
            wqa_sb = []
            for cg in range(4):
                wq_t = ph2.tile([128, 4, QLR], BF16, tag=f"wqa{cg}",
                                name="wq_t")
                src = bass.AP(tensor=wqa_d.tensor, offset=cg * 4 * 128 * QLR,
                              ap=[[QLR, 128], [128 * QLR, 4], [1, QLR]])
                nc.gpsimd.dma_start(out=wq_t, in_=src)
                wqa_sb += [wq_t[:, i, :] for i in range(4)]
            wkvb_all = interph.tile([128, NKT, HPC * (DN + DV)], BF16,
                                    tag="wkvb", name="wkvb_all")
            nc.gpsimd.dma_start(out=wkvb_all,
                                in_=_chunked(wkvb_d, NKT, 128,
                                             HPC * (DN + DV)))
            wkvb_sb = [wkvb_all[:, kt, :] for kt in range(NKT)]
            wqb_sb = []
            for cg in range(6):
                wb_t = wqbp.tile([128, 2, NH * DQK], BF16, tag=f"wqb{cg}",
                                 name="wb_t")
                src = bass.AP(tensor=wqb_d.tensor,
                              offset=cg * 2 * 128 * NH * DQK,
                              ap=[[NH * DQK, 128], [128 * NH * DQK, 2],
                                  [1, NH * DQK]])
                nc.gpsimd.dma_start(out=wb_t, in_=src)
                wqb_sb += [wb_t[:, i, :] for i in range(2)]
            wo_all = wop.tile([128, HPC, H], BF16, tag="wo", name="wo_all")
            nc.gpsimd.dma_start(out=wo_all, in_=_chunked(wo_d, HPC, 128, H))
            wo_sb = [wo_all[:, h, :] for h in range(HPC)]

            # cq: ht outer (stream wqa), both row-tiles in parallel
            s2a = ExitStack()
            ps2a = s2a.enter_context(tc.tile_pool(name="ps2a", bufs=1,
                                                  space="PSUM"))
            cq_ps = [[ps2a.tile([128, 512], F32, tag=f"cq{st}_{rb}", bufs=1,
                                name="cq_p") for rb in range(QLR // 512)]
                     for st in range(ST)]
            for ht in range(nh):
                for st in range(ST):
                    for rb in range(QLR // 512):
                        nc.tensor.matmul(out=cq_ps[st][rb],
                                         lhsT=hsT_sb[ht][:, _ts(st, 128)],
                                         rhs=wqa_sb[ht][:, _ts(rb, 512)],
                                         start=(ht == 0), stop=(ht == nh - 1))
            cqn_bf = []
            for st in range(ST):
                ssqs = []
                for rb in range(QLR // 512):
                    sqq = small.tile([128, 512], F32, tag="sqq", bufs=2,
                                     name="sqq")
                    ssq = small.tile([128, 1], F32, tag="ssq3", bufs=6,
                                     name="ssq")
                    nc.scalar.activation(out=sqq, in_=cq_ps[st][rb],
                                         func=AF.Square, accum_out=ssq)
                    ssqs.append(ssq)
                nc.vector.tensor_add(ssqs[0], ssqs[0], ssqs[1])
                nc.vector.tensor_add(ssqs[0], ssqs[0], ssqs[2])
                rstd = small.tile([128, 1], F32, tag="rstd", name="rstd")
                nc.scalar.activation(out=rstd, in_=ssqs[0], func=AF.Sqrt,
                                     scale=1.0 / QLR, bias=eps_t)
                nc.vector.reciprocal(out=rstd, in_=rstd)
                cn_t = ph2.tile([128, QLR], BF16, tag=f"cqn{st}", name="cn_t")
                for rb in range(QLR // 512):
                    nc.vector.tensor_scalar_mul(cn_t[:, _ts(rb, 512)],
                                                cq_ps[st][rb], rstd)
                cqn_bf.append(cn_t)

            # transpose cqn -> cqnT [1536, 256] (batched drains, alternating
            # engines)
            cqnT = []
            for rt in range(QLR // 128):
                cT_t = ph2.tile([128, SPC], BF16, tag=f"cqnT{rt}",
                                name="cT_t")
                p_t = ps2a.tile([128, SPC], BF16, tag="tp", bufs=2,
                                name="p_t")
                for st in range(ST):
                    nc.tensor.transpose(out=p_t[:, _ts(st, 128)],
                                        in_=cqn_bf[st][:, _ts(rt, 128)],
                                        identity=ident)
                if rt % 2 == 0:
                    nc.scalar.copy(out=cT_t, in_=p_t)
                else:
                    nc.vector.tensor_copy(out=cT_t, in_=p_t)
                cqnT.append(cT_t)
            s2a.close()

            # q_b per row-tile; psum in head-pair blocks of 384 cols so the
            # rope slicing never crosses a PSUM tile boundary
            s2b = ExitStack()
            ps2b = s2b.enter_context(tc.tile_pool(name="ps2b", bufs=1,
                                                  space="PSUM"))
            nr = QLR // 128
            for st in range(ST):
                q_ps = [ps2b.tile([128, HPC * DQK], F32, tag=f"qb{nb}",
                                  bufs=1, name="q_p") for nb in range(NC)]
                for rt in range(nr):
                    for nb in range(NC):
                        nc.tensor.matmul(out=q_ps[nb],
                                         lhsT=cqnT[rt][:, _ts(st, 128)],
                                         rhs=wqb_sb[rt][:, _ts(nb, HPC * DQK)],
                                         start=(rt == 0), stop=(rt == nr - 1))
                # rope + bf16 pack: nope copies on Act, rope muls on DVE
                q_bf = ph2.tile([128, NH, DQK], BF16, tag=f"qbf{st}",
                                name="q_bf")
                for nb in range(NC):
                    qv = q_ps[nb].rearrange("p (h d) -> p h d", h=HPC)
                    dst = q_bf[:, nb * HPC:(nb + 1) * HPC, :]
                    nc.scalar.copy(out=dst[:, :, 0:DN], in_=qv[:, :, 0:DN])

                    def _bc(t, lo, hi):
                        return bass.AP(
                            tensor=t.tensor, offset=t.offset + lo,
                            ap=[list(t.ap[0]), [0, HPC], [1, hi - lo]])
                    cs, sn = cos_sb[st], sin_sb[st]
                    xe, xo = qv[:, :, DN:DN + HW], qv[:, :, DN + HW:DQK]
                    t0 = small.tile([128, HPC, HW], F32, tag="qrs0", bufs=2,
                                    name="t0")
                    t1 = small.tile([128, HPC, HW], F32, tag="qrs1", bufs=2,
                                    name="t1")
                    nc.vector.tensor_mul(t0, xe, _bc(cs, 0, HW))
                    nc.vector.tensor_mul(t1, xo, _bc(sn, 0, HW))
                    nc.vector.tensor_sub(dst[:, :, DN:DN + HW], t0, t1)
                    nc.vector.tensor_mul(t0, xo, _bc(cs, HW, DR))
                    nc.vector.tensor_mul(t1, xe, _bc(sn, HW, DR))
                    nc.vector.tensor_add(dst[:, :, DN + HW:DQK], t0, t1)
                # stage the whole row-tile with one DMA (8 dest chunks)
                dst = bass.AP(tensor=a2aq_in.tensor,
                              offset=st * 128 * HPC * DQK,
                              ap=[[HPC * DQK, 128], [SPC * HPC * DQK, NC],
                                  [1, HPC * DQK]])
                nc.gpsimd.dma_start(out=dst, in_=q_bf)
            s2b.close()
            # ---- collective 2: AllToAll q ----
            nc.gpsimd.collective_compute(
                "AllToAll", mybir.AluOpType.bypass, replica_groups=rg,
                ins=[a2aq_in.opt()], outs=[a2aq_out.opt()])
        wqb_stack.close()

        # =========== phase 3: k/v expansion + qT ===========
        with tc.tile_pool(name="ph3", bufs=1) as ph3, \
             tc.tile_pool(name="ph3b", bufs=4) as ph3b:
            s3 = ExitStack()
            ps3 = s3.enter_context(tc.tile_pool(name="ps3", bufs=2,
                                                space="PSUM"))
            krT = ph3.tile([64, NC, SPC], BF16, tag="krT", name="krT")
            src = bass.AP(tensor=ag_out.tensor, offset=KVLR * SPC,
                          ap=[[SPC, 64], [(KVLR + DR) * SPC, NC], [1, SPC]])
            nc.sync.dma_start(out=krT, in_=src)
            krTf = krT.rearrange("p g c -> p (g c)")

            kT = [ph3.tile([128, S], BF16, tag=f"kT{h}", name="kT_t")
                  for h in range(HPC)]
            v_sb = [[ph3.tile([128, DV], BF16, tag=f"v{h}_{kc}", name="v_t")
                     for kc in range(S // 128)] for h in range(HPC)]
            for g in range(NC):
                xk_t = ph3b.tile([128, NKT, SPC], BF16, tag="xk", bufs=3,
                                 name="xk_t")
                src = bass.AP(tensor=ag_out.tensor,
                              offset=g * (KVLR + DR) * SPC,
                              ap=[[SPC, 128], [128 * SPC, NKT], [1, SPC]])
                nc.sync.dma_start(out=xk_t, in_=src)
                xk = [xk_t[:, kt, :] for kt in range(NKT)]
                kps = [ps3.tile([128, SPC], F32, tag="mmk", bufs=2, name="kp")
                       for _ in range(HPC)]
                for kt in range(NKT):
                    for h in range(HPC):
                        nc.tensor.matmul(
                            out=kps[h],
                            lhsT=wkvb_sb[kt][:, h * (DN + DV):
                                             h * (DN + DV) + DN],
                            rhs=xk[kt], start=(kt == 0), stop=(kt == NKT - 1))
                for h in range(HPC):
                    if h == 0:
                        nc.scalar.copy(out=kT[h][:, _ts(g, SPC)], in_=kps[h])
                    else:
                        nc.vector.tensor_copy(out=kT[h][:, _ts(g, SPC)],
                                              in_=kps[h])
                for sub in range(ST):
                    vps = [ps3.tile([128, DV], F32, tag="mmv", bufs=2,
                                    name="vp") for _ in range(HPC)]
                    for kt in range(NKT):
                        for h in range(HPC):
                            nc.tensor.matmul(
                                out=vps[h], lhsT=xk[kt][:, _ts(sub, 128)],
                                rhs=wkvb_sb[kt][:, h * (DN + DV) + DN:
                                                (h + 1) * (DN + DV)],
                                start=(kt == 0), stop=(kt == NKT - 1))
                    for h in range(HPC):
                        nc.vector.tensor_copy(out=v_sb[h][g * ST + sub],
                                              in_=vps[h])

            # q^T per head from the AllToAll (batched loads + drains)
            qTn = [ph3.tile([128, S], BF16, tag=f"qTn{h}", name="qTn_t")
                   for h in range(HPC)]
            qTr = [ph3.tile([64, S], BF16, tag=f"qTr{h}", name="qTr_t")
                   for h in range(HPC)]
            for q4 in range(4):
                qblk = ph3b.tile([128, 4, HPC * DQK], BF16, tag="qblk",
                                 bufs=2, name="qblk")
                src = bass.AP(tensor=a2aq_out.tensor,
                              offset=q4 * 4 * 128 * HPC * DQK,
                              ap=[[HPC * DQK, 128], [128 * HPC * DQK, 4],
                                  [1, HPC * DQK]])
                nc.sync.dma_start(out=qblk, in_=src)
                for h in range(HPC):
                    pn = ps3.tile([128, 512], BF16, tag="tqn", bufs=2,
                                  name="pn")
                    pr = ps3.tile([64, 512], BF16, tag="tqr", bufs=2,
                                  name="pr")
                    for i in range(4):
                        nc.tensor.transpose(
                            out=pn[:, _ts(i, 128)],
                            in_=qblk[:, i, h * DQK:h * DQK + DN],
                            identity=ident)
                        nc.tensor.transpose(
                            out=pr[:, _ts(i, 128)],
                            in_=qblk[:, i, h * DQK + DN:(h + 1) * DQK],
                            identity=ident)
                    if h == 0:
                        nc.scalar.copy(out=qTn[h][:, _ts(q4, 512)], in_=pn)
                        nc.vector.tensor_copy(out=qTr[h][:, _ts(q4, 512)],
                                              in_=pr)
                    else:
                        nc.vector.tensor_copy(out=qTn[h][:, _ts(q4, 512)],
                                              in_=pn)
                        nc.scalar.copy(out=qTr[h][:, _ts(q4, 512)], in_=pr)
            s3.close()

            # ====== phase 4: attention (scoresT) + interleaved o_proj ======
            QB = 512
            NQB = S // QB
            attTn = [[None] * NQB for _ in range(HPC)]

            def oproj(qb, ps_pool, o_pool):
                """o_proj for q rows [qb*512, (qb+1)*512): both heads."""
                for sub in range(4):
                    qs = qb * 4 + sub
                    o_t = o_pool.tile([128, H], BF16, tag="osb", bufs=3,
                                      name="o_t")
                    for cb in range(H // 512):
                        op = ps_pool.tile([128, 512], F32, tag="op", bufs=2,
                                          name="op")
                        for h in range(HPC):
                            nc.tensor.matmul(
                                out=op,
                                lhsT=attTn[h][qb][:, _ts(sub, 128)],
                                rhs=wo_sb[h][:, _ts(cb, 512)],
                                start=(h == 0), stop=(h == HPC - 1))
                        if cb % 2 == 0:
                            nc.scalar.copy(out=o_t[:, _ts(cb, 512)], in_=op)
                        else:
                            nc.vector.tensor_copy(out=o_t[:, _ts(cb, 512)],
                                                  in_=op)
                    nc.sync.dma_start(out=out_d[_ts(qs, 128), :], in_=o_t)

            with tc.tile_pool(name="ps5", bufs=1, space="PSUM") as ps5, \
                 tc.tile_pool(name="ph5", bufs=1) as ph5:
                for qb in range(NQB):
                    for h in range(HPC):
                        # previous block's o_proj slots between the two head
                        # chains: its inputs are long-ready, so PE streams
                        # through it with no dependency stalls
                        if h == 1 and qb > 0:
                            oproj(qb - 1, ps5, ph5)
                        attp = ps5.tile([128, QB], F32, tag="attT", bufs=2,
                                        name="attp")
                        denp = ps5.tile([1, QB], F32, tag="den", bufs=2,
                                        name="denp")
                        nkc = 4 * qb + 4
                        # software-pipelined: PV/den of kc trail the score
                        # matmuls of kc+1 so PE never waits on exp
                        probs = [None] * nkc

                        def scores(kc):
                            off = max(0, (kc - 4 * qb) * 128)
                            scp = ps5.tile([128, QB], F32, tag="scT", bufs=2,
                                           name="scp")
                            nc.tensor.matmul(
                                out=scp[:, off:QB],
                                lhsT=kT[h][:, _ts(kc, 128)],
                                rhs=qTn[h][:, qb * QB + off:(qb + 1) * QB],
                                start=True, stop=False)
                            nc.tensor.matmul(
                                out=scp[:, off:QB],
                                lhsT=krTf[:, _ts(kc, 128)],
                                rhs=qTr[h][:, qb * QB + off:(qb + 1) * QB],
                                start=False, stop=True)
                            if kc >= 4 * qb:
                                nc.vector.tensor_add(scp[:, off:off + 128],
                                                     scp[:, off:off + 128],
                                                     cmaskT)
                            pt = ph3b.tile([128, QB], BF16, tag="probsT",
                                           bufs=4, name="probsT")
                            if off > 0:
                                nc.vector.memset(pt[:, 0:off], 0.0)
                            nc.scalar.activation(out=pt[:, off:QB],
                                                 in_=scp[:, off:QB],
                                                 func=AF.Exp)
                            probs[kc] = pt

                        def pv(kc):
                            nc.tensor.matmul(out=attp, lhsT=v_sb[h][kc],
                                             rhs=probs[kc],
                                             start=(kc == 0),
                                             stop=(kc == nkc - 1))
                            nc.tensor.matmul(out=denp, lhsT=ones_bf,
                                             rhs=probs[kc],
                                             start=(kc == 0),
                                             stop=(kc == nkc - 1))

                        scores(0)
                        for kc in range(1, nkc):
                            scores(kc)
                            pv(kc - 1)
                        pv(nkc - 1)

                        # normalize while draining attT
                        rec = small.tile([1, QB], F32, tag="rec", bufs=4,
                                         name="rec")
                        nc.vector.reciprocal(out=rec, in_=denp)
                        bca = small.tile([128, QB], F32, tag="bca", bufs=2,
                                         name="bca")
                        nc.gpsimd.partition_broadcast(bca, rec)
                        a_t = ph5.tile([128, QB], BF16, tag=f"attn{h}_{qb}",
                                       name="a_t")
                        nc.vector.tensor_mul(a_t, attp, bca)
                        attTn[h][qb] = a_t
                oproj(NQB - 1, ps5, ph5)

    nc.compile()
    return nc


def _prep(hidden_states, cos, sin, wq_a, q_ln, wq_b, wkv_a, kv_ln, wkv_b, wo):
    """Host-side sharding + weight prep: pre-transpose hidden, fold layernorm
    weights + softmax scale into the B projections, pre-permute rope columns
    (de-interleave), slice wo by head, cast to bf16."""
    bf = ml_dtypes.bfloat16
    hsT = np.ascontiguousarray(hidden_states.reshape(S, H).T.astype(bf))
    cos2 = np.ascontiguousarray(cos.reshape(S, DR).astype(np.float32))
    sin2 = np.ascontiguousarray(sin.reshape(S, DR).astype(np.float32))

    # de-interleave permutation for a 64-wide rope slice
    perm = np.concatenate([np.arange(0, DR, 2), np.arange(1, DR, 2)])

    wkva = np.array(wkv_a, copy=True)
    wkva[:, KVLR:] = wkva[:, KVLR:][:, perm]
    wkva = wkva.astype(bf)

    scale = np.float32(DQK) ** np.float32(-0.5)
    wqb = np.asarray(wq_b * q_ln[:, None] * scale)
    wqb = wqb.reshape(QLR, NH, DQK)
    wqb = np.concatenate([wqb[:, :, :DN], wqb[:, :, DN:][:, :, perm]],
                         axis=2).reshape(QLR, NH * DQK).astype(bf)

    wkvb = (wkv_b * kv_ln[:, None]).astype(bf)
    wob = wo.astype(bf)

    in_maps = []
    for c in range(NC):
        r = slice(c * SPC, (c + 1) * SPC)
        hcols = slice(c * HPC * (DN + DV), (c + 1) * HPC * (DN + DV))
        hrows = slice(c * HPC * DV, (c + 1) * HPC * DV)
        in_maps.append({
            "hsT": np.ascontiguousarray(hsT[:, r]),
            "cosr": np.ascontiguousarray(cos2[r]),
            "sinr": np.ascontiguousarray(sin2[r]),
            "wqa": wq_a.astype(bf),
            "wkva": wkva,
            "wqb": wqb,
            "wkvb": np.ascontiguousarray(wkvb[:, hcols]),
            "wo": np.ascontiguousarray(wob[hrows]),
        })
    return in_maps


def kernel(**inputs) -> np.ndarray:
    if "nc" not in _CACHED:
        _CACHED["nc"] = build()
    nc = _CACHED["nc"]
    in_maps = _prep(**inputs)
    res = run_bass_kernel_spmd(nc, in_maps, list(range(NC)))
    out = np.zeros((S, H), np.float32)
    for c in range(NC):
        out += res.results[c]["out"].astype(np.float32)
    return out.reshape(B, S, H)


if __name__ == "__main__":
    rng = np.random.RandomState(0)
    ins = {
        "hidden_states": rng.randn(B, S, H).astype(np.float32),
        "cos": rng.rand(B, S, DR).astype(np.float32),
        "sin": rng.rand(B, S, DR).astype(np.float32),
        "wq_a": (rng.randn(H, QLR) * 0.02).astype(np.float32),
        "q_ln": np.ones(QLR, np.float32),
        "wq_b": (rng.randn(QLR, NH * DQK) * 0.02).astype(np.float32),
        "wkv_a": (rng.randn(H, KVLR + DR) * 0.02).astype(np.float32),
        "kv_ln": np.ones(KVLR, np.float32),
        "wkv_b": (rng.randn(KVLR, NH * (DN + DV)) * 0.02).astype(np.float32),
        "wo": (rng.randn(NH * DV, H) * 0.02).astype(np.float32),
    }
    out = kernel(**ins)
    print("kernel out", out.shape, out.dtype, np.abs(out).mean())
